# revision 10
# baseline (speedup 1.0000x reference)
"""GQA attention layer (B=1, S=2048, D=4096, H=32, KVH=8, HD=128) on 8 TRN2
NeuronCores, tensor-parallel over heads.

Each core computes 4 query heads + their shared kv head end-to-end:
QKV projection -> RoPE -> causal attention (no-max-sub softmax, scores are
tiny) -> its slice of the wo projection. The 8 partial [S, D] outputs are
summed on the host (the "all-reduce after wo" of the sharding hint).

Device layouts (bf16/fp8 into the PE, fp32 PSUM accumulation):
  QT/KT  [HD=128(part), S]    from  lhsT=w[d,:], rhs=xT[d, s-tile]
  V      [S(part), HD]        via PE-transpose of VT
  scoresT[k(part), q]         lhsT=KT chunk, rhs=QT tile
  E = exp(scoresT/128) bf16; causal diagonal via 0/1 mask multiply
  attnT  [HD(part), q]        lhsT=V chunk, rhs=E  (accumulated over k)
  denom  [1, q]               lhsT=ones[128,1], rhs=E (accumulated over k)
  attnT_norm = attnT * bcast(1/denom)   (PE outer-product broadcast)
  out    [s(part), n]         wo in fp8 DoubleRow (head pairs as the
                              doubled contraction rows); the first 128
                              output rows redone in bf16 since early
                              causal rows have the largest magnitudes.

All x/w DRAM tensors are host-pre-swizzled so every DMA reads contiguous
>=4KB lines per partition. wo weights ride the vector DMA ring with their
triggers placed after phase A so they don't steal HBM bandwidth from the
x loads.
"""

import json
import math

import ml_dtypes
import numpy as np

import concourse.bass as bass
import concourse.tile as tile
from concourse import mybir
from concourse.bass_utils import run_bass_kernel_spmd

BF16 = mybir.dt.bfloat16
F32 = mybir.dt.float32
FP8 = mybir.dt.float8e4
NPBF16 = ml_dtypes.bfloat16
NPFP8 = ml_dtypes.float8_e4m3

# Full problem constants
B, S, D = 1, 2048, 4096
H, KVH = 32, 8
HD = 128
NCORES = 8
HQ = H // NCORES  # query heads per core
MULT = 1.0
ROPE_BASE = 10000.0
ST = 512  # s-tile (PSUM bank width in fp32)


def attn_scale(seq_len=S, d_head=HD, mult=MULT):
    alpha = 1.0 / (1.0 + 4.0 * d_head / mult**2)
    lower = (math.log(seq_len) / seq_len) ** 0.5
    interp = math.exp((1.0 - alpha) * math.log(lower))
    return 1.0 / interp


def _legalize_single_wait(nc):
    """The walrus build in this container accepts only ONE sync wait per
    instruction ("Too many sync wait commands" in setupSyncWait). Split
    extra waits into preceding single-wait Drains (lowered to CTRL NOPs)
    on the same engine — same in-order stall semantics."""
    bir = json.loads(nc.to_json_bytes())
    ctr = 0
    for fn in bir["functions"]:
        for blk in fn["blocks"]:
            out = []
            for inst in blk["instructions"]:
                si = inst.get("sync_info")
                waits = (si or {}).get("on_wait") or []
                if len(waits) > 1:
                    for w in waits[:-1]:
                        ctr += 1
                        out.append(
                            {
                                "debug": inst.get("debug", 0),
                                "engine": inst["engine"],
                                "ins": [],
                                "name": f"{inst['name']}-mw{ctr}",
                                "opcode": "Drain",
                                "outs": [],
                                "sync_info": {"on_update": [], "on_wait": [w]},
                            }
                        )
                    si["on_wait"] = [waits[-1]]
                out.append(inst)
            blk["instructions"] = out
    fixed = json.dumps(bir).encode()
    nc.to_json_bytes = lambda: fixed
    return nc


def build_core_kernel(s=S, d=D, hq=HQ):
    """Bass module for one core: hq query heads + 1 kv head."""
    nst = s // ST  # s-tiles of 512
    ndk = d // 128  # contraction chunks
    nh = hq + 2  # q heads + k + v
    nnt = d // ST  # output n-tiles

    nqk = hq + 1  # q heads + k (fp8 path)
    npair = ndk // 2  # 256-row contraction pair-chunks (DoubleRow)
    nhp = hq // 2  # wo head pairs

    nc = bass.Bass()
    # host-preswizzled layouts (see host_prep): every slice a DMA pulls is
    # contiguous per partition
    x8_d = nc.dram_tensor("x8", [128, nst, 4, 4, 2, ST], FP8, kind="ExternalInput")
    xT_d = nc.dram_tensor("xT", [128, nst, 4, 8, ST], BF16, kind="ExternalInput")
    w8_d = nc.dram_tensor("w8", [8, 128, 2, 2, nqk * 128], FP8, kind="ExternalInput")
    wv_d = nc.dram_tensor("wv", [4, 128, 8, 128], BF16, kind="ExternalInput")
    wo8_d = nc.dram_tensor("wo8", [nhp, 128, 2, d], FP8, kind="ExternalInput")
    woT_d = nc.dram_tensor("woT", [hq * 128, d], BF16, kind="ExternalInput")
    cosF_d = nc.dram_tensor("cosF", [128, s], BF16, kind="ExternalInput")
    sinSg_d = nc.dram_tensor("sinSg", [128, s], BF16, kind="ExternalInput")
    maskT_d = nc.dram_tensor("maskT", [128, 128], BF16, kind="ExternalInput")
    ident_d = nc.dram_tensor("ident", [128, 128], BF16, kind="ExternalInput")
    onesc_d = nc.dram_tensor("onesc", [128, 1], BF16, kind="ExternalInput")
    onesr_d = nc.dram_tensor("onesr", [128, 128], BF16, kind="ExternalInput")
    outp_d = nc.dram_tensor("outp", [s, d], BF16, kind="ExternalOutput")

    with tile.TileContext(nc) as tc:
        with (
            tc.tile_pool(name="const", bufs=1) as cp,
            tc.tile_pool(name="qkvsb", bufs=1) as qp,
            tc.tile_pool(name="xp", bufs=3) as xp,
            tc.tile_pool(name="rp", bufs=2) as rp,
            tc.tile_pool(name="vp", bufs=2) as vp,
            tc.tile_pool(name="ep", bufs=18) as ep,
            tc.tile_pool(name="sp", bufs=2) as sp,
            tc.tile_pool(name="op", bufs=5) as op,
            tc.tile_pool(name="at", bufs=2) as atp,
        ):
            # ---- resident constants (gpsimd DMA ring, needed-first order) ----
            w8g = [
                cp.tile([128, 2, 2, nqk * 128], FP8, tag=f"w8{g}", name=f"w8{g}")
                for g in range(8)
            ]
            wvsb4 = [
                cp.tile([128, 8, 128], BF16, tag=f"wv{g}", name=f"wv{g}")
                for g in range(4)
            ]
            for g4 in range(4):
                nc.gpsimd.dma_start(w8g[2 * g4], w8_d[2 * g4])
                nc.gpsimd.dma_start(w8g[2 * g4 + 1], w8_d[2 * g4 + 1])
                nc.gpsimd.dma_start(wvsb4[g4], wv_d[g4])
            w8 = [w8g[j // 2][:, j % 2] for j in range(npair)]
            wvsb = [wvsb4[dk // 8][:, dk % 8, :] for dk in range(ndk)]
            cossb = cp.tile([128, s], BF16)
            nc.gpsimd.dma_start(cossb, cosF_d[:])
            sinsb = cp.tile([128, s], BF16)
            nc.gpsimd.dma_start(sinsb, sinSg_d[:])
            masksb = cp.tile([128, 128], BF16)
            nc.gpsimd.dma_start(masksb, maskT_d[:])
            identsb = cp.tile([128, 128], BF16)
            nc.gpsimd.dma_start(identsb, ident_d[:])
            onescsb = cp.tile([128, 1], BF16)
            nc.gpsimd.dma_start(onescsb, onesc_d[:])
            onescbsb = cp.tile([128, 128], BF16)
            nc.gpsimd.dma_start(onescbsb, onesr_d[:])
            # wo weights: tiles declared here, DMAs deferred to phase B entry
            wo8sb = [
                cp.tile([128, 2, d], FP8, tag=f"wo8{hp}", name=f"wo8{hp}")
                for hp in range(nhp)
            ]
            wosb = [
                cp.tile([128, d], BF16, tag=f"wo{mh}", name=f"wo{mh}")
                for mh in range(hq)
            ]

            # ---- persistent activations (bf16), one tile per s-tile so a
            # q-tile's attention only depends on the s-tiles it reads ----
            qts_sb = [
                [
                    qp.tile([128, ST], BF16, tag=f"QT{h}_{st}", name=f"QT{h}_{st}")
                    for st in range(nst)
                ]
                for h in range(hq)
            ]
            kts_sb = [
                qp.tile([128, ST], BF16, tag=f"KT{st}", name=f"KT{st}")
                for st in range(nst)
            ]
            vs_sb = [
                qp.tile([128, ST], BF16, tag=f"V{st}", name=f"V{st}")
                for st in range(nst)
            ]  # [s%128 part, (s//128 % 4)*HD]

            # ================= phase A: QKV projection + RoPE =================
            with (
                tc.tile_pool(name="psA", bufs=7, space="PSUM") as psA,
                tc.tile_pool(name="psT", bufs=1, space="PSUM") as psT,
            ):
                nq = 4  # quarters per s-tile
                ndkq = ndk // nq  # bf16 contraction chunks per quarter (V)
                npq = npair // nq  # fp8 pair-chunks per quarter (QK)

                def emit_rope(st, stg):
                    ssl = slice(st * ST, (st + 1) * ST)
                    for h in [hq] + list(range(hq)):
                        dst = qts_sb[h][st] if h < hq else kts_sb[st]
                        t1 = rp.tile([128, ST], BF16, tag="t1", name=f"t1_{st}_{h}")
                        nc.vector.tensor_mul(t1, stg[h], cossb[:, ssl])
                        tsw = rp.tile([128, ST], BF16, tag="tsw", name=f"tsw_{st}_{h}")
                        nc.vector.tensor_copy(tsw[0:64, :], stg[h][64:128, :])
                        nc.vector.tensor_copy(tsw[64:128, :], stg[h][0:64, :])
                        nc.vector.tensor_mul(tsw, tsw, sinsb[:, ssl])
                        nc.vector.tensor_add(dst, t1, tsw)

                rope_tail = None
                for st in range(nst):
                    ssl = slice(st * ST, (st + 1) * ST)
                    acc = [
                        psA.tile([128, ST], F32, tag="acc", name=f"acc{h}")
                        for h in range(nh)
                    ]
                    # heads-major over resident xT quarters: at the next s-tile
                    # boundary only acc[0] must be free for PE to proceed
                    for quar in range(nq):
                        x8a = xp.tile([128, npq, 2, ST], FP8, tag="x8")
                        nc.sync.dma_start(x8a, x8_d[:, st, quar])
                        xta = xp.tile([128, ndkq, ST], BF16, tag="xT")
                        nc.scalar.dma_start(xta, xT_d[:, st, quar])
                        # Q + K: fp8 DoubleRow, 256-deep contraction per matmul
                        for h in range(nqk):
                            for i in range(npq):
                                nc.tensor.matmul(
                                    acc[h],
                                    w8[quar * npq + i][:, :, h * 128 : (h + 1) * 128],
                                    x8a[:, i, :, :],
                                    start=(quar == 0 and i == 0),
                                    stop=(quar == nq - 1 and i == npq - 1),
                                    perf_mode=mybir.MatmulPerfMode.DoubleRow,
                                )
                        # V: bf16
                        for dk in range(ndkq):
                            nc.tensor.matmul(
                                acc[nh - 1],
                                wvsb[quar * ndkq + dk],
                                xta[:, dk, :],
                                start=(quar == 0 and dk == 0),
                                stop=(quar == nq - 1 and dk == ndkq - 1),
                            )
                    # V: transpose [HD, s-tile] -> [s-chunk, HD] blocks.
                    # Emitted before RoPE so the PE's transposes don't wait
                    # behind the RoPE chain on the DVE.
                    for j in range(ST // 128):
                        vtmp = vp.tile([128, 128], BF16, tag="vtmp")
                        nc.scalar.copy(vtmp, acc[hq + 1][:, j * 128 : (j + 1) * 128])
                        tp_ps = psT.tile([128, 128], BF16, tag="tp")
                        nc.tensor.transpose(tp_ps, vtmp, identsb)
                        nc.vector.tensor_copy(
                            vs_sb[st][:, j * 128 : (j + 1) * 128], tp_ps
                        )
                    # stage QK accumulators to bf16 (one ACT copy each, frees
                    # the PSUM bank early); RoPE then runs in bf16 on the DVE
                    stg = {}
                    for h in range(hq + 1):
                        stg[h] = rp.tile(
                            [128, ST], BF16, tag=f"stg{h}", name=f"stg{st}_{h}"
                        )
                        nc.scalar.copy(stg[h], acc[h])
                    if st < nst - 1:
                        emit_rope(st, stg)
                    else:
                        rope_tail = (st, stg)

            # wo weight DMAs ride the scalar ring behind all 16 xT loads
            # (rings are FIFO), so they can't steal HBM bandwidth from
            # phase A's x loads
            for hp in range(nhp):
                nc.scalar.dma_start(wo8sb[hp], wo8_d[hp])
            for mh in range(hq):
                nc.scalar.dma_start(wosb[mh], woT_d[mh * 128 : (mh + 1) * 128, :])

            # ============ phase B: attention + output projection ============
            with (
                tc.tile_pool(name="psS", bufs=3, space="PSUM") as psS,
                tc.tile_pool(name="psD", bufs=1, space="PSUM") as psD,
                tc.tile_pool(name="psAt", bufs=4, space="PSUM") as psAt,
            ):

                def emit_wo(qt, attn8, attnb):
                    # wo for the s-chunks of q-tile qt (emitted one q-tile
                    # late so the normalize tail overlaps these matmuls).
                    # fp8 DoubleRow over head pairs; the first 128 rows
                    # (largest magnitudes, short causal span) redone in bf16.
                    with nc.named_scope(f"wo{qt}"):
                        for j in range(ST // 128):
                            sc = qt * (ST // 128) + j
                            for ntg in range(nnt // 2):
                                osb = op.tile(
                                    [128, 2 * ST], BF16, tag="osb",
                                    name=f"osb{qt}_{j}_{ntg}",
                                )
                                for half in range(2):
                                    nt = 2 * ntg + half
                                    o_ps = psS.tile(
                                        [128, ST], F32, tag="sc",
                                        name=f"wo{qt}_{j}_{nt}",
                                    )
                                    if qt == 0 and j == 0:
                                        for mh in range(hq):
                                            nc.tensor.matmul(
                                                o_ps,
                                                attnb[mh],
                                                wosb[mh][:, nt * ST : (nt + 1) * ST],
                                                start=(mh == 0),
                                                stop=(mh == hq - 1),
                                            )
                                    else:
                                        for hp in range(nhp):
                                            nc.tensor.matmul(
                                                o_ps,
                                                attn8[hp][:, :, j * 128 : (j + 1) * 128],
                                                wo8sb[hp][:, :, nt * ST : (nt + 1) * ST],
                                                start=(hp == 0),
                                                stop=(hp == nhp - 1),
                                                perf_mode=mybir.MatmulPerfMode.DoubleRow,
                                            )
                                    dst = osb[:, half * ST : (half + 1) * ST]
                                    if half == 0:
                                        nc.vector.tensor_copy(dst, o_ps)
                                    else:
                                        nc.scalar.copy(dst, o_ps)
                                # one 2KB-per-partition DMA per pair, rings
                                # alternated so out-DMA triggers don't
                                # serialize behind each other
                                ring = nc.sync if ntg % 2 == 0 else nc.gpsimd
                                ring.dma_start(
                                    outp_d[
                                        sc * 128 : (sc + 1) * 128,
                                        2 * ntg * ST : (2 * ntg + 2) * ST,
                                    ],
                                    osb,
                                )

                prev_wo = None
                for qt in range(nst):
                    nk = (qt + 1) * (ST // 128)  # causal: k chunks this q-tile
                    with nc.named_scope(f"attn{qt}"):
                        # one denominator bank per q-tile: head h accumulates
                        # into partition row 32*h (distinct col-groups)
                        den4 = psD.tile([128, ST], F32, tag="den")
                        nc.vector.memset(den4, 1.0)
                        at_tiles = {
                            h: psAt.tile([128, ST], F32, tag="at", name=f"at{qt}_{h}")
                            for h in range(hq)
                        }
                        av_defer = []
                        for c in range(nk):
                            # diagonal chunks: only columns >= 128*r valid
                            r = c - (nk - 4)
                            off = 128 * r if r > 0 else 0
                            w = ST - off
                            e_ts = {}
                            for h in range(hq):
                                sc_ps = psS.tile(
                                    [128, ST], F32, tag="sc", name=f"sc{qt}_{c}_{h}"
                                )
                                nc.tensor.matmul(
                                    sc_ps[:, 0:w],
                                    kts_sb[c // 4][:, (c % 4) * 128 : (c % 4 + 1) * 128],
                                    qts_sb[h][qt][:, off:ST],
                                    start=True,
                                    stop=True,
                                )
                                e_t = ep.tile(
                                    [128, ST], BF16, tag="E", name=f"e{qt}_{c}_{h}"
                                )
                                nc.scalar.activation(
                                    e_t[:, 0:w],
                                    sc_ps[:, 0:w],
                                    mybir.ActivationFunctionType.Exp,
                                    scale=1.0 / HD,
                                )
                                if r >= 0:
                                    nc.vector.tensor_mul(
                                        e_t[:, 0:128], e_t[:, 0:128], masksb
                                    )
                                e_ts[h] = e_t
                            if qt == 0:
                                av_defer.append((c, off, w, e_ts))
                            else:
                                for h in range(hq):
                                    nc.tensor.matmul(
                                        at_tiles[h][:, off:ST],
                                        vs_sb[c // 4][:, (c % 4) * 128 : (c % 4 + 1) * 128],
                                        e_ts[h][:, 0:w],
                                        start=(c == 0),
                                        stop=(c == nk - 1),
                                    )
                            # 4 single-row denominator matmuls in distinct
                            # col-groups
                            for h in range(hq):
                                nc.tensor.matmul(
                                    den4[32 * h : 32 * h + 1, off:ST],
                                    onescsb,
                                    e_ts[h][:, 0:w],
                                    start=(c == 0),
                                    stop=(c == nk - 1),
                                    tile_position=(0, 32 * h),
                                )
                        for c, off, w, e_ts in av_defer:
                            for h in range(hq):
                                nc.tensor.matmul(
                                    at_tiles[h][:, off:ST],
                                    vs_sb[c // 4][:, (c % 4) * 128 : (c % 4 + 1) * 128],
                                    e_ts[h][:, 0:w],
                                    start=(c == 0),
                                    stop=(c == nk - 1),
                                )
                    # previous q-tile's wo matmuls fill the PE while this
                    # q-tile's reciprocal+broadcast tail runs on ACT/DVE
                    if prev_wo is not None:
                        emit_wo(*prev_wo)
                    with nc.named_scope(f"norm{qt}"):
                        # strided reciprocal for all 4 heads' denominators;
                        # approx gives 18 bits (we round to bf16 anyway) at
                        # ~5x less DVE time, and den4 is in [1, ~3e3] so the
                        # undefined edge cases can't occur
                        recip = sp.tile([128, ST], F32, tag="recip", name=f"recip{qt}")
                        nc.vector.reciprocal(recip, den4)
                        recipb = sp.tile([128, ST], BF16, tag="recipb", name=f"recipb{qt}")
                        nc.scalar.copy(recipb, recip)
                        attn8 = [
                            atp.tile([128, 2, ST], FP8, tag=f"attn8{hp}",
                                     name=f"attn8_{qt}_{hp}")
                            for hp in range(nhp)
                        ]
                        attnb = None
                        if qt == 0:
                            attnb = [
                                atp.tile([128, 128], BF16, tag=f"attnb{mh}",
                                         name=f"attnb{mh}")
                                for mh in range(hq)
                            ]
                        for hh in range(hq):
                            bc_ps = psS.tile(
                                [128, ST], F32, tag="sc", name=f"bc{qt}_{hh}"
                            )
                            nc.tensor.matmul(
                                bc_ps,
                                onescbsb[32 * hh : 32 * hh + 1, :],
                                recipb[32 * hh : 32 * hh + 1, :],
                                start=True,
                                stop=True,
                                tile_position=(32 * hh, 0),
                            )
                            bc_sb = sp.tile(
                                [128, ST], F32, tag="bcsb", name=f"bcsb{qt}_{hh}"
                            )
                            nc.scalar.copy(bc_sb, bc_ps)
                            nc.vector.tensor_mul(
                                attn8[hh // 2][:, hh % 2, :], at_tiles[hh], bc_sb
                            )
                            if qt == 0:
                                nc.vector.tensor_mul(
                                    attnb[hh], at_tiles[hh][:, 0:128],
                                    bc_sb[:, 0:128],
                                )
                    prev_wo = (qt, attn8, attnb)
                    if qt == 0 and rope_tail is not None:
                        emit_rope(*rope_tail)
                emit_wo(*prev_wo)
    return _legalize_single_wait(nc)


def host_prep(x, wq, wk, wv, wo, s=S, d=D, hq=HQ, ncores=NCORES):
    """Shared tensors + per-core weight shards, all host-side numpy.
    Everything is pre-swizzled into the exact on-chip tile layouts so each
    DMA slice reads contiguous per-partition lines."""
    scale = attn_scale(s, HD, MULT)
    nst = s // ST
    xTf = np.ascontiguousarray(x.reshape(s, d).T)  # [d, s]
    # xT [p, st, quar, dk8, n] <- xT[(quar*8+dk8)*128+p, st*512+n]
    xT = np.ascontiguousarray(
        xTf.astype(NPBF16).reshape(4, 8, 128, nst, ST).transpose(2, 3, 0, 1, 4)
    )
    # x8 [p, st, quar, i4, ko, n] <- x8[(quar*4+i4)*256+ko*128+p, st*512+n]
    x8 = np.ascontiguousarray(
        xTf.astype(NPFP8).reshape(4, 4, 2, 128, nst, ST).transpose(3, 4, 0, 1, 2, 5)
    )

    freq = ROPE_BASE ** (-(np.arange(0, HD, 2, dtype=np.float64) / HD))
    pos = np.arange(s, dtype=np.float64)
    angle = pos[:, None] * freq[None, :]  # [s, 64]
    cos = np.cos(angle).astype(NPBF16).T  # [64, s]
    sin = np.sin(angle).astype(NPBF16).T
    cosF = np.ascontiguousarray(np.concatenate([cos, cos], axis=0))
    sinSg = np.ascontiguousarray(np.concatenate([-sin, sin], axis=0))

    # triangular causal mask for diagonal chunks: keep iff p <= f
    p = np.arange(128)[:, None]
    f = np.arange(128)[None, :]
    maskT = (p <= f).astype(NPBF16)  # [128, 128]

    ident = np.eye(128, dtype=NPBF16)
    onesc = np.ones((128, 1), dtype=NPBF16)
    onesr = np.ones((128, 128), dtype=NPBF16)

    shared = dict(
        xT=xT, x8=x8, cosF=cosF, sinSg=sinSg, maskT=maskT, ident=ident,
        onesc=onesc, onesr=onesr,
    )

    in_maps = []
    for c in range(ncores):
        wq_c = wq[c * hq * 128 : (c + 1) * hq * 128, :]  # [hq*128, d]
        wk_c = wk[c * 128 : (c + 1) * 128, :]
        wv_c = wv[c * 128 : (c + 1) * 128, :] * scale
        wqk8f = np.ascontiguousarray(
            np.concatenate([wq_c.T, wk_c.T], axis=1)
        ).astype(NPFP8)  # [d, (hq+1)*128]
        # w8 [g, p, j2, ko, m] <- wqk8[((g*2+j2)*2+ko)*128+p, m]
        w8 = np.ascontiguousarray(
            wqk8f.reshape(8, 2, 2, 128, (hq + 1) * 128).transpose(0, 3, 1, 2, 4)
        )
        # wv [g, p, c8, n] <- wvT[(g*8+c8)*128+p, n]
        wvT = np.ascontiguousarray(
            wv_c.T.astype(NPBF16).reshape(4, 8, 128, 128).transpose(0, 2, 1, 3)
        )
        wo_c = wo[:, c * hq * 128 : (c + 1) * hq * 128]  # [d, hq*128]
        woT = np.ascontiguousarray(wo_c.T)  # [hq*128, d] f32
        # wo8 [hp, ki, ko, n] <- woT[(2hp+ko)*128+ki, n]
        wo8 = np.ascontiguousarray(
            woT.astype(NPFP8).reshape(hq // 2, 2, 128, d).transpose(0, 2, 1, 3)
        )
        in_maps.append(
            dict(shared, w8=w8, wv=wvT, wo8=wo8, woT=woT.astype(NPBF16))
        )
    return in_maps


_NC_CACHE = {}


def kernel(x, freqs_cis, wq, wk, wv, wo):
    del freqs_cis  # forward pass recomputes rope tables (matches reference)
    x = np.asarray(x, dtype=np.float32)
    key = (S, D, HQ)
    if key not in _NC_CACHE:
        _NC_CACHE[key] = build_core_kernel(S, D, HQ)
    nc = _NC_CACHE[key]
    in_maps = host_prep(
        x, np.asarray(wq, np.float32), np.asarray(wk, np.float32),
        np.asarray(wv, np.float32), np.asarray(wo, np.float32),
    )
    res = run_bass_kernel_spmd(nc, in_maps, core_ids=list(range(NCORES)))
    out = np.zeros((S, D), dtype=np.float32)
    for r in res.results:
        out += np.asarray(r["outp"], dtype=np.float32)
    return out.reshape(B, S, D)


if __name__ == "__main__":
    rng = np.random.default_rng(0)
    x = rng.standard_normal((B, S, D)).astype(np.float32)
    wq = (rng.standard_normal((H * HD, D)) * D**-0.5).astype(np.float32)
    wk = (rng.standard_normal((KVH * HD, D)) * D**-0.5).astype(np.float32)
    wv = (rng.standard_normal((KVH * HD, D)) * D**-0.5).astype(np.float32)
    wo = (rng.standard_normal((D, H * HD)) * (H * HD) ** -0.5).astype(np.float32)
    fc = rng.standard_normal((S, HD // 2)).astype(np.float32)
    out = kernel(x, fc, wq, wk, wv, wo)
    print(out.shape, out.dtype, np.abs(out).max())


# revision 11
# speedup vs baseline: 1.0596x; 1.0596x over previous
"""GQA attention layer (B=1, S=2048, D=4096, H=32, KVH=8, HD=128) on 8 TRN2
NeuronCores, tensor-parallel over heads.

Each core computes 4 query heads + their shared kv head end-to-end:
QKV projection -> RoPE -> causal attention (no-max-sub softmax, scores are
tiny) -> its slice of the wo projection. The 8 partial [S, D] outputs are
summed on the host (the "all-reduce after wo" of the sharding hint).

Device layouts (bf16/fp8 into the PE, fp32 PSUM accumulation):
  QT/KT  [HD=128(part), S]    from  lhsT=w[d,:], rhs=xT[d, s-tile]
  V      [S(part), HD]        via PE-transpose of VT
  scoresT[k(part), q]         lhsT=KT chunk, rhs=QT tile
  E = exp(scoresT/128) bf16; causal diagonal via 0/1 mask multiply
  attnT  [HD(part), q]        lhsT=V chunk, rhs=E  (accumulated over k)
  denom  [1, q]               lhsT=ones[128,1], rhs=E (accumulated over k)
  attnT_norm = attnT * bcast(1/denom)   (PE outer-product broadcast)
  out    [s(part), n]         wo in fp8 DoubleRow (head pairs as the
                              doubled contraction rows); the first 128
                              output rows redone in bf16 since early
                              causal rows have the largest magnitudes.

All x/w DRAM tensors are host-pre-swizzled so every DMA reads contiguous
>=4KB lines per partition. wo weights ride the vector DMA ring with their
triggers placed after phase A so they don't steal HBM bandwidth from the
x loads.
"""

import json
import math

import ml_dtypes
import numpy as np

import concourse.bass as bass
import concourse.tile as tile
from concourse import mybir
from concourse.bass_utils import run_bass_kernel_spmd

BF16 = mybir.dt.bfloat16
F32 = mybir.dt.float32
FP8 = mybir.dt.float8e4
NPBF16 = ml_dtypes.bfloat16
NPFP8 = ml_dtypes.float8_e4m3

# Full problem constants
B, S, D = 1, 2048, 4096
H, KVH = 32, 8
HD = 128
NCORES = 8
HQ = H // NCORES  # query heads per core
MULT = 1.0
ROPE_BASE = 10000.0
ST = 512  # s-tile (PSUM bank width in fp32)


def attn_scale(seq_len=S, d_head=HD, mult=MULT):
    alpha = 1.0 / (1.0 + 4.0 * d_head / mult**2)
    lower = (math.log(seq_len) / seq_len) ** 0.5
    interp = math.exp((1.0 - alpha) * math.log(lower))
    return 1.0 / interp


def _legalize_single_wait(nc):
    """The walrus build in this container accepts only ONE sync wait per
    instruction ("Too many sync wait commands" in setupSyncWait). Split
    extra waits into preceding single-wait Drains (lowered to CTRL NOPs)
    on the same engine — same in-order stall semantics."""
    bir = json.loads(nc.to_json_bytes())
    ctr = 0
    for fn in bir["functions"]:
        for blk in fn["blocks"]:
            out = []
            for inst in blk["instructions"]:
                si = inst.get("sync_info")
                waits = (si or {}).get("on_wait") or []
                if len(waits) > 1:
                    for w in waits[:-1]:
                        ctr += 1
                        out.append(
                            {
                                "debug": inst.get("debug", 0),
                                "engine": inst["engine"],
                                "ins": [],
                                "name": f"{inst['name']}-mw{ctr}",
                                "opcode": "Drain",
                                "outs": [],
                                "sync_info": {"on_update": [], "on_wait": [w]},
                            }
                        )
                    si["on_wait"] = [waits[-1]]
                out.append(inst)
            blk["instructions"] = out
    fixed = json.dumps(bir).encode()
    nc.to_json_bytes = lambda: fixed
    return nc


def build_core_kernel(s=S, d=D, hq=HQ):
    """Bass module for one core: hq query heads + 1 kv head."""
    nst = s // ST  # s-tiles of 512
    ndk = d // 128  # contraction chunks
    nh = hq + 2  # q heads + k + v
    nnt = d // ST  # output n-tiles

    nqk = hq + 1  # q heads + k (fp8 path)
    npair = ndk // 2  # 256-row contraction pair-chunks (DoubleRow)
    nhp = hq // 2  # wo head pairs

    nc = bass.Bass()
    # host-preswizzled layouts (see host_prep): every slice a DMA pulls is
    # contiguous per partition
    x8_d = nc.dram_tensor("x8", [128, nst, 4, 4, 2, ST], FP8, kind="ExternalInput")
    xT_d = nc.dram_tensor("xT", [128, nst, 4, 8, ST], BF16, kind="ExternalInput")
    w8_d = nc.dram_tensor("w8", [8, 128, 2, 2, nqk * 128], FP8, kind="ExternalInput")
    wv_d = nc.dram_tensor("wv", [4, 128, 8, 128], BF16, kind="ExternalInput")
    wo8_d = nc.dram_tensor("wo8", [nhp, 128, 2, d], FP8, kind="ExternalInput")
    woT_d = nc.dram_tensor("woT", [hq * 128, d], BF16, kind="ExternalInput")
    cosF_d = nc.dram_tensor("cosF", [128, s], BF16, kind="ExternalInput")
    sinSg_d = nc.dram_tensor("sinSg", [128, s], BF16, kind="ExternalInput")
    maskT_d = nc.dram_tensor("maskT", [128, 128], BF16, kind="ExternalInput")
    ident_d = nc.dram_tensor("ident", [128, 128], BF16, kind="ExternalInput")
    onesc_d = nc.dram_tensor("onesc", [128, 1], BF16, kind="ExternalInput")
    onesr_d = nc.dram_tensor("onesr", [128, 128], BF16, kind="ExternalInput")
    outp_d = nc.dram_tensor("outp", [s, d], BF16, kind="ExternalOutput")

    with tile.TileContext(nc) as tc:
        with (
            tc.tile_pool(name="const", bufs=1) as cp,
            tc.tile_pool(name="qkvsb", bufs=1) as qp,
            tc.tile_pool(name="xp", bufs=3) as xp,
            tc.tile_pool(name="rp", bufs=2) as rp,
            tc.tile_pool(name="vp", bufs=2) as vp,
            tc.tile_pool(name="ep", bufs=18) as ep,
            tc.tile_pool(name="sp", bufs=2) as sp,
            tc.tile_pool(name="op", bufs=5) as op,
            tc.tile_pool(name="at", bufs=2) as atp,
        ):
            # ---- resident constants (gpsimd DMA ring, needed-first order) ----
            w8g = [
                cp.tile([128, 2, 2, nqk * 128], FP8, tag=f"w8{g}", name=f"w8{g}")
                for g in range(8)
            ]
            wvsb4 = [
                cp.tile([128, 8, 128], BF16, tag=f"wv{g}", name=f"wv{g}")
                for g in range(4)
            ]
            for g4 in range(4):
                nc.gpsimd.dma_start(w8g[2 * g4], w8_d[2 * g4])
                nc.gpsimd.dma_start(w8g[2 * g4 + 1], w8_d[2 * g4 + 1])
                nc.gpsimd.dma_start(wvsb4[g4], wv_d[g4])
            w8 = [w8g[j // 2][:, j % 2] for j in range(npair)]
            wvsb = [wvsb4[dk // 8][:, dk % 8, :] for dk in range(ndk)]
            cossb = cp.tile([128, s], BF16)
            nc.gpsimd.dma_start(cossb, cosF_d[:])
            sinsb = cp.tile([128, s], BF16)
            nc.gpsimd.dma_start(sinsb, sinSg_d[:])
            masksb = cp.tile([128, 128], BF16)
            nc.gpsimd.dma_start(masksb, maskT_d[:])
            identsb = cp.tile([128, 128], BF16)
            nc.gpsimd.dma_start(identsb, ident_d[:])
            onescsb = cp.tile([128, 1], BF16)
            nc.gpsimd.dma_start(onescsb, onesc_d[:])
            onescbsb = cp.tile([128, 128], BF16)
            nc.gpsimd.dma_start(onescbsb, onesr_d[:])
            # wo weights: tiles declared here, DMAs deferred to phase B entry
            wo8sb = [
                cp.tile([128, 2, d], FP8, tag=f"wo8{hp}", name=f"wo8{hp}")
                for hp in range(nhp)
            ]
            wosb = [
                cp.tile([128, d], BF16, tag=f"wo{mh}", name=f"wo{mh}")
                for mh in range(hq)
            ]

            # ---- persistent activations (bf16), one tile per s-tile so a
            # q-tile's attention only depends on the s-tiles it reads ----
            qts_sb = [
                [
                    qp.tile([128, ST], BF16, tag=f"QT{h}_{st}", name=f"QT{h}_{st}")
                    for st in range(nst)
                ]
                for h in range(hq)
            ]
            kts_sb = [
                qp.tile([128, ST], BF16, tag=f"KT{st}", name=f"KT{st}")
                for st in range(nst)
            ]
            vs_sb = [
                qp.tile([128, ST], BF16, tag=f"V{st}", name=f"V{st}")
                for st in range(nst)
            ]  # [s%128 part, (s//128 % 4)*HD]

            # ================= phase A: QKV projection + RoPE =================
            with (
                tc.tile_pool(name="psA", bufs=7, space="PSUM") as psA,
                tc.tile_pool(name="psT", bufs=1, space="PSUM") as psT,
            ):
                nq = 4  # quarters per s-tile
                ndkq = ndk // nq  # bf16 contraction chunks per quarter (V)
                npq = npair // nq  # fp8 pair-chunks per quarter (QK)

                def emit_rope(st, stg):
                    ssl = slice(st * ST, (st + 1) * ST)
                    for h in [hq] + list(range(hq)):
                        dst = qts_sb[h][st] if h < hq else kts_sb[st]
                        t1 = rp.tile([128, ST], BF16, tag="t1", name=f"t1_{st}_{h}")
                        nc.vector.tensor_mul(t1, stg[h], cossb[:, ssl])
                        tsw = rp.tile([128, ST], BF16, tag="tsw", name=f"tsw_{st}_{h}")
                        nc.vector.tensor_copy(tsw[0:64, :], stg[h][64:128, :])
                        nc.vector.tensor_copy(tsw[64:128, :], stg[h][0:64, :])
                        nc.vector.tensor_mul(tsw, tsw, sinsb[:, ssl])
                        nc.vector.tensor_add(dst, t1, tsw)

                rope_tail = None
                for st in range(nst):
                    ssl = slice(st * ST, (st + 1) * ST)
                    acc = [
                        psA.tile([128, ST], F32, tag="acc", name=f"acc{h}")
                        for h in range(nh)
                    ]
                    # heads-major over resident xT quarters: at the next s-tile
                    # boundary only acc[0] must be free for PE to proceed
                    for quar in range(nq):
                        x8a = xp.tile([128, npq, 2, ST], FP8, tag="x8")
                        nc.sync.dma_start(x8a, x8_d[:, st, quar])
                        xta = xp.tile([128, ndkq, ST], BF16, tag="xT")
                        nc.scalar.dma_start(xta, xT_d[:, st, quar])
                        # Q + K: fp8 DoubleRow, 256-deep contraction per matmul
                        for h in range(nqk):
                            for i in range(npq):
                                nc.tensor.matmul(
                                    acc[h],
                                    w8[quar * npq + i][:, :, h * 128 : (h + 1) * 128],
                                    x8a[:, i, :, :],
                                    start=(quar == 0 and i == 0),
                                    stop=(quar == nq - 1 and i == npq - 1),
                                    perf_mode=mybir.MatmulPerfMode.DoubleRow,
                                )
                        # V: bf16
                        for dk in range(ndkq):
                            nc.tensor.matmul(
                                acc[nh - 1],
                                wvsb[quar * ndkq + dk],
                                xta[:, dk, :],
                                start=(quar == 0 and dk == 0),
                                stop=(quar == nq - 1 and dk == ndkq - 1),
                            )
                    # V: transpose [HD, s-tile] -> [s-chunk, HD] blocks.
                    # Emitted before RoPE so the PE's transposes don't wait
                    # behind the RoPE chain on the DVE.
                    for j in range(ST // 128):
                        vtmp = vp.tile([128, 128], BF16, tag="vtmp")
                        nc.scalar.copy(vtmp, acc[hq + 1][:, j * 128 : (j + 1) * 128])
                        tp_ps = psT.tile([128, 128], BF16, tag="tp")
                        nc.tensor.transpose(tp_ps, vtmp, identsb)
                        nc.vector.tensor_copy(
                            vs_sb[st][:, j * 128 : (j + 1) * 128], tp_ps
                        )
                    # stage QK accumulators to bf16 (one ACT copy each, frees
                    # the PSUM bank early); RoPE then runs in bf16 on the DVE
                    stg = {}
                    for h in range(hq + 1):
                        stg[h] = rp.tile(
                            [128, ST], BF16, tag=f"stg{h}", name=f"stg{st}_{h}"
                        )
                        nc.scalar.copy(stg[h], acc[h])
                    if st < nst - 1:
                        emit_rope(st, stg)
                    else:
                        rope_tail = (st, stg)

            # wo weight DMAs ride the scalar ring behind all 16 xT loads
            # (rings are FIFO), so they can't steal HBM bandwidth from
            # phase A's x loads
            for hp in range(nhp):
                nc.scalar.dma_start(wo8sb[hp], wo8_d[hp])
            for mh in range(hq):
                nc.scalar.dma_start(wosb[mh], woT_d[mh * 128 : (mh + 1) * 128, :])

            # ============ phase B: attention + output projection ============
            with (
                tc.tile_pool(name="psS", bufs=3, space="PSUM") as psS,
                tc.tile_pool(name="psD", bufs=1, space="PSUM") as psD,
                tc.tile_pool(name="psAt", bufs=4, space="PSUM") as psAt,
            ):

                def emit_wo(qt, attn8, attnb, lo=0, hi=16):
                    # wo units [lo:hi) for q-tile qt; a unit is one s-chunk x
                    # two n-tiles -> one 2KB-per-partition out DMA. Units are
                    # interleaved into the NEXT q-tile's attention chunks so
                    # the out-DMA bandwidth is spread across the whole window
                    # instead of bursting (8 cores bursting saturates HBM
                    # writes). fp8 DoubleRow over head pairs; the first 128
                    # rows (largest magnitudes, short span) redone in bf16.
                    # Both halves of a pair are copied by the SAME engine so
                    # the DMA has a single producer; engines and DMA rings
                    # alternate per unit.
                    with nc.named_scope(f"wo{qt}"):
                        for u in range(lo, hi):
                            j, ntg = u // (nnt // 2), u % (nnt // 2)
                            sc = qt * (ST // 128) + j
                            osb = op.tile(
                                [128, 2 * ST], BF16, tag="osb",
                                name=f"osb{qt}_{j}_{ntg}",
                            )
                            eng = nc.vector if u % 2 == 0 else nc.scalar
                            for half in range(2):
                                nt = 2 * ntg + half
                                o_ps = psS.tile(
                                    [128, ST], F32, tag="sc",
                                    name=f"wo{qt}_{j}_{nt}",
                                )
                                if qt == 0 and j == 0:
                                    for mh in range(hq):
                                        nc.tensor.matmul(
                                            o_ps,
                                            attnb[mh],
                                            wosb[mh][:, nt * ST : (nt + 1) * ST],
                                            start=(mh == 0),
                                            stop=(mh == hq - 1),
                                        )
                                else:
                                    for hp in range(nhp):
                                        nc.tensor.matmul(
                                            o_ps,
                                            attn8[hp][:, :, j * 128 : (j + 1) * 128],
                                            wo8sb[hp][:, :, nt * ST : (nt + 1) * ST],
                                            start=(hp == 0),
                                            stop=(hp == nhp - 1),
                                            perf_mode=mybir.MatmulPerfMode.DoubleRow,
                                        )
                                dst = osb[:, half * ST : (half + 1) * ST]
                                if u % 2 == 0:
                                    nc.vector.tensor_copy(dst, o_ps)
                                else:
                                    nc.scalar.copy(dst, o_ps)
                            ring = nc.sync if u % 2 == 0 else nc.gpsimd
                            ring.dma_start(
                                outp_d[
                                    sc * 128 : (sc + 1) * 128,
                                    2 * ntg * ST : (2 * ntg + 2) * ST,
                                ],
                                osb,
                            )

                prev_wo = None
                for qt in range(nst):
                    nk = (qt + 1) * (ST // 128)  # causal: k chunks this q-tile
                    emitted = 0
                    with nc.named_scope(f"attn{qt}"):
                        # one denominator bank per q-tile: head h accumulates
                        # into partition row 32*h (distinct col-groups)
                        den4 = psD.tile([128, ST], F32, tag="den")
                        nc.vector.memset(den4, 1.0)
                        at_tiles = {
                            h: psAt.tile([128, ST], F32, tag="at", name=f"at{qt}_{h}")
                            for h in range(hq)
                        }
                        av_defer = []
                        for c in range(nk):
                            # diagonal chunks: only columns >= 128*r valid
                            r = c - (nk - 4)
                            off = 128 * r if r > 0 else 0
                            w = ST - off
                            e_ts = {}
                            for h in range(hq):
                                sc_ps = psS.tile(
                                    [128, ST], F32, tag="sc", name=f"sc{qt}_{c}_{h}"
                                )
                                nc.tensor.matmul(
                                    sc_ps[:, 0:w],
                                    kts_sb[c // 4][:, (c % 4) * 128 : (c % 4 + 1) * 128],
                                    qts_sb[h][qt][:, off:ST],
                                    start=True,
                                    stop=True,
                                )
                                e_t = ep.tile(
                                    [128, ST], BF16, tag="E", name=f"e{qt}_{c}_{h}"
                                )
                                nc.scalar.activation(
                                    e_t[:, 0:w],
                                    sc_ps[:, 0:w],
                                    mybir.ActivationFunctionType.Exp,
                                    scale=1.0 / HD,
                                )
                                if r >= 0:
                                    nc.vector.tensor_mul(
                                        e_t[:, 0:128], e_t[:, 0:128], masksb
                                    )
                                e_ts[h] = e_t
                            if qt == 0:
                                av_defer.append((c, off, w, e_ts))
                            else:
                                for h in range(hq):
                                    nc.tensor.matmul(
                                        at_tiles[h][:, off:ST],
                                        vs_sb[c // 4][:, (c % 4) * 128 : (c % 4 + 1) * 128],
                                        e_ts[h][:, 0:w],
                                        start=(c == 0),
                                        stop=(c == nk - 1),
                                    )
                            # 4 single-row denominator matmuls in distinct
                            # col-groups
                            for h in range(hq):
                                nc.tensor.matmul(
                                    den4[32 * h : 32 * h + 1, off:ST],
                                    onescsb,
                                    e_ts[h][:, 0:w],
                                    start=(c == 0),
                                    stop=(c == nk - 1),
                                    tile_position=(0, 32 * h),
                                )
                            # spread 12 of prev's 16 wo units across this
                            # q-tile's chunks; hold 4 back to cover the
                            # reciprocal latency after the last denominator
                            if prev_wo is not None and c < nk - 1:
                                target = (12 * (c + 1)) // nk
                                if target > emitted:
                                    emit_wo(*prev_wo, emitted, target)
                                    emitted = target
                        for c, off, w, e_ts in av_defer:
                            for h in range(hq):
                                nc.tensor.matmul(
                                    at_tiles[h][:, off:ST],
                                    vs_sb[c // 4][:, (c % 4) * 128 : (c % 4 + 1) * 128],
                                    e_ts[h][:, 0:w],
                                    start=(c == 0),
                                    stop=(c == nk - 1),
                                )
                    # previous q-tile's remaining wo matmuls fill the PE
                    # while this q-tile's reciprocal tail runs on the DVE
                    if prev_wo is not None:
                        emit_wo(*prev_wo, emitted, 16)
                    with nc.named_scope(f"norm{qt}"):
                        # strided reciprocal for all 4 heads' denominators;
                        # approx gives 18 bits (we round to bf16 anyway) at
                        # ~5x less DVE time, and den4 is in [1, ~3e3] so the
                        # undefined edge cases can't occur
                        recip = sp.tile([128, ST], F32, tag="recip", name=f"recip{qt}")
                        nc.vector.reciprocal(recip, den4)
                        recipb = sp.tile([128, ST], BF16, tag="recipb", name=f"recipb{qt}")
                        nc.scalar.copy(recipb, recip)
                        attn8 = [
                            atp.tile([128, 2, ST], FP8, tag=f"attn8{hp}",
                                     name=f"attn8_{qt}_{hp}")
                            for hp in range(nhp)
                        ]
                        attnb = None
                        if qt == 0:
                            attnb = [
                                atp.tile([128, 128], BF16, tag=f"attnb{mh}",
                                         name=f"attnb{mh}")
                                for mh in range(hq)
                            ]
                        for hh in range(hq):
                            bc_ps = psS.tile(
                                [128, ST], F32, tag="sc", name=f"bc{qt}_{hh}"
                            )
                            nc.tensor.matmul(
                                bc_ps,
                                onescbsb[32 * hh : 32 * hh + 1, :],
                                recipb[32 * hh : 32 * hh + 1, :],
                                start=True,
                                stop=True,
                                tile_position=(32 * hh, 0),
                            )
                            bc_sb = sp.tile(
                                [128, ST], F32, tag="bcsb", name=f"bcsb{qt}_{hh}"
                            )
                            nc.scalar.copy(bc_sb, bc_ps)
                            nc.vector.tensor_mul(
                                attn8[hh // 2][:, hh % 2, :], at_tiles[hh], bc_sb
                            )
                            if qt == 0:
                                nc.vector.tensor_mul(
                                    attnb[hh], at_tiles[hh][:, 0:128],
                                    bc_sb[:, 0:128],
                                )
                    prev_wo = (qt, attn8, attnb)
                    if qt == 0 and rope_tail is not None:
                        emit_rope(*rope_tail)
                emit_wo(*prev_wo)
    return _legalize_single_wait(nc)


def host_prep(x, wq, wk, wv, wo, s=S, d=D, hq=HQ, ncores=NCORES):
    """Shared tensors + per-core weight shards, all host-side numpy.
    Everything is pre-swizzled into the exact on-chip tile layouts so each
    DMA slice reads contiguous per-partition lines."""
    scale = attn_scale(s, HD, MULT)
    nst = s // ST
    xTf = np.ascontiguousarray(x.reshape(s, d).T)  # [d, s]
    # xT [p, st, quar, dk8, n] <- xT[(quar*8+dk8)*128+p, st*512+n]
    xT = np.ascontiguousarray(
        xTf.astype(NPBF16).reshape(4, 8, 128, nst, ST).transpose(2, 3, 0, 1, 4)
    )
    # x8 [p, st, quar, i4, ko, n] <- x8[(quar*4+i4)*256+ko*128+p, st*512+n]
    x8 = np.ascontiguousarray(
        xTf.astype(NPFP8).reshape(4, 4, 2, 128, nst, ST).transpose(3, 4, 0, 1, 2, 5)
    )

    freq = ROPE_BASE ** (-(np.arange(0, HD, 2, dtype=np.float64) / HD))
    pos = np.arange(s, dtype=np.float64)
    angle = pos[:, None] * freq[None, :]  # [s, 64]
    cos = np.cos(angle).astype(NPBF16).T  # [64, s]
    sin = np.sin(angle).astype(NPBF16).T
    cosF = np.ascontiguousarray(np.concatenate([cos, cos], axis=0))
    sinSg = np.ascontiguousarray(np.concatenate([-sin, sin], axis=0))

    # triangular causal mask for diagonal chunks: keep iff p <= f
    p = np.arange(128)[:, None]
    f = np.arange(128)[None, :]
    maskT = (p <= f).astype(NPBF16)  # [128, 128]

    ident = np.eye(128, dtype=NPBF16)
    onesc = np.ones((128, 1), dtype=NPBF16)
    onesr = np.ones((128, 128), dtype=NPBF16)

    shared = dict(
        xT=xT, x8=x8, cosF=cosF, sinSg=sinSg, maskT=maskT, ident=ident,
        onesc=onesc, onesr=onesr,
    )

    in_maps = []
    for c in range(ncores):
        wq_c = wq[c * hq * 128 : (c + 1) * hq * 128, :]  # [hq*128, d]
        wk_c = wk[c * 128 : (c + 1) * 128, :]
        wv_c = wv[c * 128 : (c + 1) * 128, :] * scale
        wqk8f = np.ascontiguousarray(
            np.concatenate([wq_c.T, wk_c.T], axis=1)
        ).astype(NPFP8)  # [d, (hq+1)*128]
        # w8 [g, p, j2, ko, m] <- wqk8[((g*2+j2)*2+ko)*128+p, m]
        w8 = np.ascontiguousarray(
            wqk8f.reshape(8, 2, 2, 128, (hq + 1) * 128).transpose(0, 3, 1, 2, 4)
        )
        # wv [g, p, c8, n] <- wvT[(g*8+c8)*128+p, n]
        wvT = np.ascontiguousarray(
            wv_c.T.astype(NPBF16).reshape(4, 8, 128, 128).transpose(0, 2, 1, 3)
        )
        wo_c = wo[:, c * hq * 128 : (c + 1) * hq * 128]  # [d, hq*128]
        woT = np.ascontiguousarray(wo_c.T)  # [hq*128, d] f32
        # wo8 [hp, ki, ko, n] <- woT[(2hp+ko)*128+ki, n]
        wo8 = np.ascontiguousarray(
            woT.astype(NPFP8).reshape(hq // 2, 2, 128, d).transpose(0, 2, 1, 3)
        )
        in_maps.append(
            dict(shared, w8=w8, wv=wvT, wo8=wo8, woT=woT.astype(NPBF16))
        )
    return in_maps


_NC_CACHE = {}


def kernel(x, freqs_cis, wq, wk, wv, wo):
    del freqs_cis  # forward pass recomputes rope tables (matches reference)
    x = np.asarray(x, dtype=np.float32)
    key = (S, D, HQ)
    if key not in _NC_CACHE:
        _NC_CACHE[key] = build_core_kernel(S, D, HQ)
    nc = _NC_CACHE[key]
    in_maps = host_prep(
        x, np.asarray(wq, np.float32), np.asarray(wk, np.float32),
        np.asarray(wv, np.float32), np.asarray(wo, np.float32),
    )
    res = run_bass_kernel_spmd(nc, in_maps, core_ids=list(range(NCORES)))
    out = np.zeros((S, D), dtype=np.float32)
    for r in res.results:
        out += np.asarray(r["outp"], dtype=np.float32)
    return out.reshape(B, S, D)


if __name__ == "__main__":
    rng = np.random.default_rng(0)
    x = rng.standard_normal((B, S, D)).astype(np.float32)
    wq = (rng.standard_normal((H * HD, D)) * D**-0.5).astype(np.float32)
    wk = (rng.standard_normal((KVH * HD, D)) * D**-0.5).astype(np.float32)
    wv = (rng.standard_normal((KVH * HD, D)) * D**-0.5).astype(np.float32)
    wo = (rng.standard_normal((D, H * HD)) * (H * HD) ** -0.5).astype(np.float32)
    fc = rng.standard_normal((S, HD // 2)).astype(np.float32)
    out = kernel(x, fc, wq, wk, wv, wo)
    print(out.shape, out.dtype, np.abs(out).max())


# revision 14
# speedup vs baseline: 1.1456x; 1.0812x over previous
"""GQA attention layer (B=1, S=2048, D=4096, H=32, KVH=8, HD=128) on 8 TRN2
NeuronCores, tensor-parallel over heads.

Each core computes 4 query heads + their shared kv head end-to-end:
QKV projection -> RoPE -> causal attention (no-max-sub softmax, scores are
tiny) -> its slice of the wo projection. The 8 partial [S, D] outputs are
summed on the host (the "all-reduce after wo" of the sharding hint).

Device layouts (bf16/fp8 into the PE, fp32 PSUM accumulation):
  QT/KT  [HD=128(part), S]    from  lhsT=w[d,:], rhs=xT[d, s-tile]
  V      [S(part), HD]        via PE-transpose of VT
  scoresT[k(part), q]         lhsT=KT chunk, rhs=QT tile
  E = exp(scoresT/128) bf16; causal diagonal via 0/1 mask multiply
  attnT  [HD(part), q]        lhsT=V chunk, rhs=E  (accumulated over k)
  denom  [1, q]               lhsT=ones[128,1], rhs=E (accumulated over k)
  attnT_norm = attnT * bcast(1/denom)   (PE outer-product broadcast)
  out    [s(part), n]         wo in fp8 DoubleRow (head pairs as the
                              doubled contraction rows); the first 128
                              output rows redone in bf16 since early
                              causal rows have the largest magnitudes.

All x/w DRAM tensors are host-pre-swizzled so every DMA reads contiguous
>=4KB lines per partition. wo weights ride the vector DMA ring with their
triggers placed after phase A so they don't steal HBM bandwidth from the
x loads.
"""

import json
import math

import ml_dtypes
import numpy as np

import concourse.bass as bass
import concourse.tile as tile
from concourse import mybir
from concourse.bass_utils import run_bass_kernel_spmd

BF16 = mybir.dt.bfloat16
F32 = mybir.dt.float32
FP8 = mybir.dt.float8e4
NPBF16 = ml_dtypes.bfloat16
NPFP8 = ml_dtypes.float8_e4m3

# Full problem constants
B, S, D = 1, 2048, 4096
H, KVH = 32, 8
HD = 128
NCORES = 8
HQ = H // NCORES  # query heads per core
MULT = 1.0
ROPE_BASE = 10000.0
ST = 512  # s-tile (PSUM bank width in fp32)


def attn_scale(seq_len=S, d_head=HD, mult=MULT):
    alpha = 1.0 / (1.0 + 4.0 * d_head / mult**2)
    lower = (math.log(seq_len) / seq_len) ** 0.5
    interp = math.exp((1.0 - alpha) * math.log(lower))
    return 1.0 / interp


def _legalize_single_wait(nc):
    """The walrus build in this container accepts only ONE sync wait per
    instruction ("Too many sync wait commands" in setupSyncWait). Split
    extra waits into preceding single-wait Drains (lowered to CTRL NOPs)
    on the same engine — same in-order stall semantics."""
    bir = json.loads(nc.to_json_bytes())
    ctr = 0
    for fn in bir["functions"]:
        for blk in fn["blocks"]:
            out = []
            for inst in blk["instructions"]:
                si = inst.get("sync_info")
                waits = (si or {}).get("on_wait") or []
                if len(waits) > 1:
                    for w in waits[:-1]:
                        ctr += 1
                        out.append(
                            {
                                "debug": inst.get("debug", 0),
                                "engine": inst["engine"],
                                "ins": [],
                                "name": f"{inst['name']}-mw{ctr}",
                                "opcode": "Drain",
                                "outs": [],
                                "sync_info": {"on_update": [], "on_wait": [w]},
                            }
                        )
                    si["on_wait"] = [waits[-1]]
                out.append(inst)
            blk["instructions"] = out
    fixed = json.dumps(bir).encode()
    nc.to_json_bytes = lambda: fixed
    return nc


def build_core_kernel(s=S, d=D, hq=HQ):
    """Bass module for one core: hq query heads + 1 kv head."""
    nst = s // ST  # s-tiles of 512
    ndk = d // 128  # contraction chunks
    nh = hq + 2  # q heads + k + v
    nnt = d // ST  # output n-tiles

    nqk = hq + 1  # q heads + k (rope'd)
    nqkv = hq + 2  # q + k + v, all through the fp8 DoubleRow path
    npair = ndk // 2  # 256-row contraction pair-chunks (DoubleRow)
    nhp = hq // 2  # wo head pairs

    nc = bass.Bass()
    # host-preswizzled layouts (see host_prep): every slice a DMA pulls is
    # contiguous per partition
    x8_d = nc.dram_tensor("x8", [128, nst, 4, 4, 2, ST], FP8, kind="ExternalInput")
    xT_d = nc.dram_tensor("xT", [128, 4, 8, ST], BF16, kind="ExternalInput")
    w8_d = nc.dram_tensor("w8", [8, 128, 2, 2, nqkv * 128], FP8, kind="ExternalInput")
    wv_d = nc.dram_tensor("wv", [4, 128, 8, 128], BF16, kind="ExternalInput")
    wo8_d = nc.dram_tensor("wo8", [nhp, 128, 2, d], FP8, kind="ExternalInput")
    woT_d = nc.dram_tensor("woT", [hq * 128, d], BF16, kind="ExternalInput")
    cosF_d = nc.dram_tensor("cosF", [128, s], BF16, kind="ExternalInput")
    sinSg_d = nc.dram_tensor("sinSg", [128, s], BF16, kind="ExternalInput")
    maskT_d = nc.dram_tensor("maskT", [128, 128], BF16, kind="ExternalInput")
    ident_d = nc.dram_tensor("ident", [128, 128], BF16, kind="ExternalInput")
    onesc_d = nc.dram_tensor("onesc", [128, 1], BF16, kind="ExternalInput")
    onesr_d = nc.dram_tensor("onesr", [128, 128], BF16, kind="ExternalInput")
    outp_d = nc.dram_tensor("outp", [s, d], BF16, kind="ExternalOutput")

    with tile.TileContext(nc) as tc:
        with (
            tc.tile_pool(name="const", bufs=1) as cp,
            tc.tile_pool(name="qkvsb", bufs=1) as qp,
            tc.tile_pool(name="xp", bufs=3) as xp,
            tc.tile_pool(name="xb", bufs=2) as xb,
            tc.tile_pool(name="rp", bufs=2) as rp,
            tc.tile_pool(name="vp", bufs=2) as vp,
            tc.tile_pool(name="ep", bufs=18) as ep,
            tc.tile_pool(name="sp", bufs=2) as sp,
            tc.tile_pool(name="op", bufs=5) as op,
            tc.tile_pool(name="at", bufs=2) as atp,
        ):
            # ---- resident constants (gpsimd DMA ring, needed-first order) ----
            w8g = [
                cp.tile([128, 2, 2, nqkv * 128], FP8, tag=f"w8{g}", name=f"w8{g}")
                for g in range(8)
            ]
            wvsb4 = [
                cp.tile([128, 8, 128], BF16, tag=f"wv{g}", name=f"wv{g}")
                for g in range(4)
            ]
            for g4 in range(4):
                nc.gpsimd.dma_start(w8g[2 * g4], w8_d[2 * g4])
                nc.gpsimd.dma_start(w8g[2 * g4 + 1], w8_d[2 * g4 + 1])
                nc.gpsimd.dma_start(wvsb4[g4], wv_d[g4])
            w8 = [w8g[j // 2][:, j % 2] for j in range(npair)]
            wvsb = [wvsb4[dk // 8][:, dk % 8, :] for dk in range(ndk)]
            cossb = cp.tile([128, s], BF16)
            nc.gpsimd.dma_start(cossb, cosF_d[:])
            sinsb = cp.tile([128, s], BF16)
            nc.gpsimd.dma_start(sinsb, sinSg_d[:])
            masksb = cp.tile([128, 128], BF16)
            nc.gpsimd.dma_start(masksb, maskT_d[:])
            identsb = cp.tile([128, 128], BF16)
            nc.gpsimd.dma_start(identsb, ident_d[:])
            onescsb = cp.tile([128, 1], BF16)
            nc.gpsimd.dma_start(onescsb, onesc_d[:])
            onescbsb = cp.tile([128, 128], BF16)
            nc.gpsimd.dma_start(onescbsb, onesr_d[:])
            # wo weights: tiles declared here, DMAs deferred to phase B entry
            wo8sb = [
                cp.tile([128, 2, d], FP8, tag=f"wo8{hp}", name=f"wo8{hp}")
                for hp in range(nhp)
            ]
            wosb = [
                cp.tile([128, d], BF16, tag=f"wo{mh}", name=f"wo{mh}")
                for mh in range(hq)
            ]

            warm = cp.tile([128, ST], BF16, tag="warm")
            nc.vector.memset(warm, 0.0)

            # ---- persistent activations (bf16), one tile per s-tile so a
            # q-tile's attention only depends on the s-tiles it reads ----
            qts_sb = [
                [
                    qp.tile([128, ST], BF16, tag=f"QT{h}_{st}", name=f"QT{h}_{st}")
                    for st in range(nst)
                ]
                for h in range(hq)
            ]
            kts_sb = [
                qp.tile([128, ST], BF16, tag=f"KT{st}", name=f"KT{st}")
                for st in range(nst)
            ]
            vs_sb = [
                qp.tile([128, ST], BF16, tag=f"V{st}", name=f"V{st}")
                for st in range(nst)
            ]  # [s%128 part, (s//128 % 4)*HD]

            # ================= phase A: QKV projection + RoPE =================
            with (
                tc.tile_pool(name="psA", bufs=7, space="PSUM") as psA,
                tc.tile_pool(name="psT", bufs=1, space="PSUM") as psT,
            ):
                nq = 4  # quarters per s-tile
                ndkq = ndk // nq  # bf16 contraction chunks per quarter (V)
                npq = npair // nq  # fp8 pair-chunks per quarter (QK)

                def emit_rope(st, stg):
                    ssl = slice(st * ST, (st + 1) * ST)
                    for h in [hq] + list(range(hq)):
                        dst = qts_sb[h][st] if h < hq else kts_sb[st]
                        t1 = rp.tile([128, ST], BF16, tag="t1", name=f"t1_{st}_{h}")
                        nc.vector.tensor_mul(t1, stg[h], cossb[:, ssl])
                        tsw = rp.tile([128, ST], BF16, tag="tsw", name=f"tsw_{st}_{h}")
                        nc.vector.tensor_copy(tsw[0:64, :], stg[h][64:128, :])
                        nc.vector.tensor_copy(tsw[64:128, :], stg[h][0:64, :])
                        nc.vector.tensor_mul(tsw, tsw, sinsb[:, ssl])
                        nc.vector.tensor_add(dst, t1, tsw)

                rope_tail = None
                # warmup matmuls on a zeroed tile: no input deps, so they run
                # during the initial DMA wait, keeping the PE busy from ~6us
                # and the HAM clock-gate at full rate when real work lands
                wm_ps = psA.tile([128, ST], F32, tag="acc", name="warmps")
                for _ in range(44):
                    nc.tensor.matmul(wm_ps, warm[:, 0:128], warm, start=True, stop=True)
                for st in range(nst):
                    ssl = slice(st * ST, (st + 1) * ST)
                    acc = [
                        psA.tile([128, ST], F32, tag="acc", name=f"acc{h}")
                        for h in range(nqkv)
                    ]
                    if st == 0:
                        acc_vb = psA.tile([128, ST], F32, tag="acc", name="accvb")
                    # heads-major over resident xT quarters: at the next s-tile
                    # boundary only acc[0] must be free for PE to proceed
                    for quar in range(nq):
                        x8a = xp.tile([128, npq, 2, ST], FP8, tag="x8")
                        if st == 0 and quar == 0:
                            # split the very first load so the first matmul's
                            # slice lands sooner
                            nc.sync.dma_start(x8a[:, 0:2], x8_d[:, st, quar, 0:2])
                            nc.sync.dma_start(x8a[:, 2:4], x8_d[:, st, quar, 2:4])
                        else:
                            nc.sync.dma_start(x8a, x8_d[:, st, quar])
                        # Q + K + V: fp8 DoubleRow, 256-deep contraction per mm
                        for h in range(nqkv):
                            for i in range(npq):
                                nc.tensor.matmul(
                                    acc[h],
                                    w8[quar * npq + i][:, :, h * 128 : (h + 1) * 128],
                                    x8a[:, i, :, :],
                                    start=(quar == 0 and i == 0),
                                    stop=(quar == nq - 1 and i == npq - 1),
                                    perf_mode=mybir.MatmulPerfMode.DoubleRow,
                                )
                        if st == 0:
                            # accurate bf16 V for the first s-tile: its rows
                            # feed the first 512 output rows, where magnitudes
                            # are largest and the fp8 V error would show
                            xta = xb.tile([128, ndkq, ST], BF16, tag="xT")
                            nc.scalar.dma_start(xta, xT_d[:, quar])
                            for dk in range(ndkq):
                                nc.tensor.matmul(
                                    acc_vb,
                                    wvsb[quar * ndkq + dk],
                                    xta[:, dk, :],
                                    start=(quar == 0 and dk == 0),
                                    stop=(quar == nq - 1 and dk == ndkq - 1),
                                )
                    # V: transpose [HD, s-tile] -> [s-chunk, HD] blocks.
                    # Emitted before RoPE so the PE's transposes don't wait
                    # behind the RoPE chain on the DVE.
                    for j in range(ST // 128):
                        vtmp = vp.tile([128, 128], BF16, tag="vtmp")
                        nc.scalar.copy(vtmp, (acc_vb if st == 0 else acc[hq + 1])[:, j * 128 : (j + 1) * 128])
                        tp_ps = psT.tile([128, 128], BF16, tag="tp")
                        nc.tensor.transpose(tp_ps, vtmp, identsb)
                        nc.vector.tensor_copy(
                            vs_sb[st][:, j * 128 : (j + 1) * 128], tp_ps
                        )
                    # stage QK accumulators to bf16 (one ACT copy each, frees
                    # the PSUM bank early); RoPE then runs in bf16 on the DVE
                    stg = {}
                    for h in range(hq + 1):
                        stg[h] = rp.tile(
                            [128, ST], BF16, tag=f"stg{h}", name=f"stg{st}_{h}"
                        )
                        nc.scalar.copy(stg[h], acc[h])
                    if st < nst - 1:
                        emit_rope(st, stg)
                    else:
                        rope_tail = (st, stg)

            # wo weight DMAs ride the scalar ring behind all 16 xT loads
            # (rings are FIFO), so they can't steal HBM bandwidth from
            # phase A's x loads
            for hp in range(nhp):
                nc.scalar.dma_start(wo8sb[hp], wo8_d[hp])
            for mh in range(hq):
                nc.scalar.dma_start(wosb[mh], woT_d[mh * 128 : (mh + 1) * 128, :])

            # ============ phase B: attention + output projection ============
            with (
                tc.tile_pool(name="psS", bufs=3, space="PSUM") as psS,
                tc.tile_pool(name="psD", bufs=1, space="PSUM") as psD,
                tc.tile_pool(name="psAt", bufs=4, space="PSUM") as psAt,
            ):

                def emit_wo(qt, attn8, attnb, lo=0, hi=16):
                    # wo units [lo:hi) for q-tile qt; a unit is one s-chunk x
                    # two n-tiles -> one 2KB-per-partition out DMA. Units are
                    # interleaved into the NEXT q-tile's attention chunks so
                    # the out-DMA bandwidth is spread across the whole window
                    # instead of bursting (8 cores bursting saturates HBM
                    # writes). fp8 DoubleRow over head pairs; the first 128
                    # rows (largest magnitudes, short span) redone in bf16.
                    # Both halves of a pair are copied by the SAME engine so
                    # the DMA has a single producer; engines and DMA rings
                    # alternate per unit.
                    with nc.named_scope(f"wo{qt}"):
                        for u in range(lo, hi):
                            j, ntg = u // (nnt // 2), u % (nnt // 2)
                            sc = qt * (ST // 128) + j
                            osb = op.tile(
                                [128, 2 * ST], BF16, tag="osb",
                                name=f"osb{qt}_{j}_{ntg}",
                            )

                            for half in range(2):
                                nt = 2 * ntg + half
                                o_ps = psS.tile(
                                    [128, ST], F32, tag="sc",
                                    name=f"wo{qt}_{j}_{nt}",
                                )
                                if qt == 0 and j == 0:
                                    for mh in range(hq):
                                        nc.tensor.matmul(
                                            o_ps,
                                            attnb[mh],
                                            wosb[mh][:, nt * ST : (nt + 1) * ST],
                                            start=(mh == 0),
                                            stop=(mh == hq - 1),
                                        )
                                else:
                                    for hp in range(nhp):
                                        nc.tensor.matmul(
                                            o_ps,
                                            attn8[hp][:, :, j * 128 : (j + 1) * 128],
                                            wo8sb[hp][:, :, nt * ST : (nt + 1) * ST],
                                            start=(hp == 0),
                                            stop=(hp == nhp - 1),
                                            perf_mode=mybir.MatmulPerfMode.DoubleRow,
                                        )
                                dst = osb[:, half * ST : (half + 1) * ST]
                                if u % 4 != 3:
                                    nc.vector.tensor_copy(dst, o_ps)
                                else:
                                    nc.scalar.copy(dst, o_ps)
                            ring = nc.sync if u % 4 != 3 else nc.gpsimd
                            ring.dma_start(
                                outp_d[
                                    sc * 128 : (sc + 1) * 128,
                                    2 * ntg * ST : (2 * ntg + 2) * ST,
                                ],
                                osb,
                            )

                prev_wo = None
                for qt in range(nst):
                    nk = (qt + 1) * (ST // 128)  # causal: k chunks this q-tile
                    emitted = 0
                    with nc.named_scope(f"attn{qt}"):
                        # one denominator bank per q-tile: head h accumulates
                        # into partition row 32*h (distinct col-groups)
                        den4 = psD.tile([128, ST], F32, tag="den")
                        nc.vector.memset(den4, 1.0)
                        at_tiles = {
                            h: psAt.tile([128, ST], F32, tag="at", name=f"at{qt}_{h}")
                            for h in range(hq)
                        }
                        av_defer = []
                        for c in range(nk):
                            # diagonal chunks: only columns >= 128*r valid
                            r = c - (nk - 4)
                            off = 128 * r if r > 0 else 0
                            w = ST - off
                            e_ts = {}
                            for h in range(hq):
                                sc_ps = psS.tile(
                                    [128, ST], F32, tag="sc", name=f"sc{qt}_{c}_{h}"
                                )
                                nc.tensor.matmul(
                                    sc_ps[:, 0:w],
                                    kts_sb[c // 4][:, (c % 4) * 128 : (c % 4 + 1) * 128],
                                    qts_sb[h][qt][:, off:ST],
                                    start=True,
                                    stop=True,
                                )
                                e_t = ep.tile(
                                    [128, ST], BF16, tag="E", name=f"e{qt}_{c}_{h}"
                                )
                                nc.scalar.activation(
                                    e_t[:, 0:w],
                                    sc_ps[:, 0:w],
                                    mybir.ActivationFunctionType.Exp,
                                    scale=1.0 / HD,
                                )
                                if r >= 0:
                                    nc.vector.tensor_mul(
                                        e_t[:, 0:128], e_t[:, 0:128], masksb
                                    )
                                e_ts[h] = e_t
                            if qt == 0:
                                av_defer.append((c, off, w, e_ts))
                            else:
                                for h in range(hq):
                                    nc.tensor.matmul(
                                        at_tiles[h][:, off:ST],
                                        vs_sb[c // 4][:, (c % 4) * 128 : (c % 4 + 1) * 128],
                                        e_ts[h][:, 0:w],
                                        start=(c == 0),
                                        stop=(c == nk - 1),
                                    )
                            # 4 single-row denominator matmuls in distinct
                            # col-groups
                            for h in range(hq):
                                nc.tensor.matmul(
                                    den4[32 * h : 32 * h + 1, off:ST],
                                    onescsb,
                                    e_ts[h][:, 0:w],
                                    start=(c == 0),
                                    stop=(c == nk - 1),
                                    tile_position=(0, 32 * h),
                                )
                            # spread 12 of prev's 16 wo units across this
                            # q-tile's chunks; hold 4 back to cover the
                            # reciprocal latency after the last denominator
                            if prev_wo is not None and c < nk - 1:
                                target = (12 * (c + 1)) // nk
                                if target > emitted:
                                    emit_wo(*prev_wo, emitted, target)
                                    emitted = target
                        for c, off, w, e_ts in av_defer:
                            for h in range(hq):
                                nc.tensor.matmul(
                                    at_tiles[h][:, off:ST],
                                    vs_sb[c // 4][:, (c % 4) * 128 : (c % 4 + 1) * 128],
                                    e_ts[h][:, 0:w],
                                    start=(c == 0),
                                    stop=(c == nk - 1),
                                )
                    # previous q-tile's remaining wo matmuls fill the PE
                    # while this q-tile's reciprocal tail runs on the DVE
                    if prev_wo is not None:
                        emit_wo(*prev_wo, emitted, 16)
                    with nc.named_scope(f"norm{qt}"):
                        # reciprocal as exp(-ln(x)) on the Scalar engine:
                        # ~1.2us latency instead of 3.4us on the DVE, and no
                        # DVE-queue coupling; den4 is in [1, ~3e3] so ln is
                        # well-conditioned and bf16 output absorbs the error
                        lnden = sp.tile([128, ST], F32, tag="recip", name=f"lnden{qt}")
                        nc.scalar.activation(
                            lnden, den4, mybir.ActivationFunctionType.Ln
                        )
                        recipb = sp.tile([128, ST], BF16, tag="recipb", name=f"recipb{qt}")
                        nc.scalar.activation(
                            recipb, lnden, mybir.ActivationFunctionType.Exp,
                            scale=-1.0,
                        )
                        attn8 = [
                            atp.tile([128, 2, ST], FP8, tag=f"attn8{hp}",
                                     name=f"attn8_{qt}_{hp}")
                            for hp in range(nhp)
                        ]
                        attnb = None
                        if qt == 0:
                            attnb = [
                                atp.tile([128, 128], BF16, tag=f"attnb{mh}",
                                         name=f"attnb{mh}")
                                for mh in range(hq)
                            ]
                        for hh in range(hq):
                            bc_ps = psS.tile(
                                [128, ST], F32, tag="sc", name=f"bc{qt}_{hh}"
                            )
                            nc.tensor.matmul(
                                bc_ps,
                                onescbsb[32 * hh : 32 * hh + 1, :],
                                recipb[32 * hh : 32 * hh + 1, :],
                                start=True,
                                stop=True,
                                tile_position=(32 * hh, 0),
                            )
                            bc_sb = sp.tile(
                                [128, ST], F32, tag="bcsb", name=f"bcsb{qt}_{hh}"
                            )
                            nc.scalar.copy(bc_sb, bc_ps)
                            nc.vector.tensor_mul(
                                attn8[hh // 2][:, hh % 2, :], at_tiles[hh], bc_sb
                            )
                            if qt == 0:
                                nc.vector.tensor_mul(
                                    attnb[hh], at_tiles[hh][:, 0:128],
                                    bc_sb[:, 0:128],
                                )
                    prev_wo = (qt, attn8, attnb)
                    if qt == 0 and rope_tail is not None:
                        emit_rope(*rope_tail)
                emit_wo(*prev_wo)
    return _legalize_single_wait(nc)


def host_prep(x, wq, wk, wv, wo, s=S, d=D, hq=HQ, ncores=NCORES):
    """Shared tensors + per-core weight shards, all host-side numpy.
    Everything is pre-swizzled into the exact on-chip tile layouts so each
    DMA slice reads contiguous per-partition lines."""
    scale = attn_scale(s, HD, MULT)
    nst = s // ST
    xTf = np.ascontiguousarray(x.reshape(s, d).T)  # [d, s]
    # xT (s-tile 0 only, for the accurate bf16 V of the first 512 rows)
    # [p, quar, dk8, n] <- xT[(quar*8+dk8)*128+p, n]
    xT = np.ascontiguousarray(
        xTf[:, 0:ST].astype(NPBF16).reshape(4, 8, 128, ST).transpose(2, 0, 1, 3)
    )
    # x8 [p, st, quar, i4, ko, n] <- x8[(quar*4+i4)*256+ko*128+p, st*512+n]
    x8 = np.ascontiguousarray(
        xTf.astype(NPFP8).reshape(4, 4, 2, 128, nst, ST).transpose(3, 4, 0, 1, 2, 5)
    )

    freq = ROPE_BASE ** (-(np.arange(0, HD, 2, dtype=np.float64) / HD))
    pos = np.arange(s, dtype=np.float64)
    angle = pos[:, None] * freq[None, :]  # [s, 64]
    cos = np.cos(angle).astype(NPBF16).T  # [64, s]
    sin = np.sin(angle).astype(NPBF16).T
    cosF = np.ascontiguousarray(np.concatenate([cos, cos], axis=0))
    sinSg = np.ascontiguousarray(np.concatenate([-sin, sin], axis=0))

    # triangular causal mask for diagonal chunks: keep iff p <= f
    p = np.arange(128)[:, None]
    f = np.arange(128)[None, :]
    maskT = (p <= f).astype(NPBF16)  # [128, 128]

    ident = np.eye(128, dtype=NPBF16)
    onesc = np.ones((128, 1), dtype=NPBF16)
    onesr = np.ones((128, 128), dtype=NPBF16)

    shared = dict(
        xT=xT, x8=x8, cosF=cosF, sinSg=sinSg, maskT=maskT, ident=ident,
        onesc=onesc, onesr=onesr,
    )

    in_maps = []
    for c in range(ncores):
        wq_c = wq[c * hq * 128 : (c + 1) * hq * 128, :]  # [hq*128, d]
        wk_c = wk[c * 128 : (c + 1) * 128, :]
        wv_c = wv[c * 128 : (c + 1) * 128, :] * scale
        wqk8f = np.ascontiguousarray(
            np.concatenate([wq_c.T, wk_c.T, wv_c.T], axis=1)
        ).astype(NPFP8)  # [d, (hq+2)*128]
        # w8 [g, p, j2, ko, m] <- wqk8[((g*2+j2)*2+ko)*128+p, m]
        w8 = np.ascontiguousarray(
            wqk8f.reshape(8, 2, 2, 128, (hq + 2) * 128).transpose(0, 3, 1, 2, 4)
        )
        # wv [g, p, c8, n] <- wvT[(g*8+c8)*128+p, n]
        wvT = np.ascontiguousarray(
            wv_c.T.astype(NPBF16).reshape(4, 8, 128, 128).transpose(0, 2, 1, 3)
        )
        wo_c = wo[:, c * hq * 128 : (c + 1) * hq * 128]  # [d, hq*128]
        woT = np.ascontiguousarray(wo_c.T)  # [hq*128, d] f32
        # wo8 [hp, ki, ko, n] <- woT[(2hp+ko)*128+ki, n]
        wo8 = np.ascontiguousarray(
            woT.astype(NPFP8).reshape(hq // 2, 2, 128, d).transpose(0, 2, 1, 3)
        )
        in_maps.append(
            dict(shared, w8=w8, wv=wvT, wo8=wo8, woT=woT.astype(NPBF16))
        )
    return in_maps


_NC_CACHE = {}


def kernel(x, freqs_cis, wq, wk, wv, wo):
    del freqs_cis  # forward pass recomputes rope tables (matches reference)
    x = np.asarray(x, dtype=np.float32)
    key = (S, D, HQ)
    if key not in _NC_CACHE:
        _NC_CACHE[key] = build_core_kernel(S, D, HQ)
    nc = _NC_CACHE[key]
    in_maps = host_prep(
        x, np.asarray(wq, np.float32), np.asarray(wk, np.float32),
        np.asarray(wv, np.float32), np.asarray(wo, np.float32),
    )
    res = run_bass_kernel_spmd(nc, in_maps, core_ids=list(range(NCORES)))
    out = np.zeros((S, D), dtype=np.float32)
    for r in res.results:
        out += np.asarray(r["outp"], dtype=np.float32)
    return out.reshape(B, S, D)


if __name__ == "__main__":
    rng = np.random.default_rng(0)
    x = rng.standard_normal((B, S, D)).astype(np.float32)
    wq = (rng.standard_normal((H * HD, D)) * D**-0.5).astype(np.float32)
    wk = (rng.standard_normal((KVH * HD, D)) * D**-0.5).astype(np.float32)
    wv = (rng.standard_normal((KVH * HD, D)) * D**-0.5).astype(np.float32)
    wo = (rng.standard_normal((D, H * HD)) * (H * HD) ** -0.5).astype(np.float32)
    fc = rng.standard_normal((S, HD // 2)).astype(np.float32)
    out = kernel(x, fc, wq, wk, wv, wo)
    print(out.shape, out.dtype, np.abs(out).max())


# revision 17
# speedup vs baseline: 1.1738x; 1.0246x over previous
"""GQA attention layer (B=1, S=2048, D=4096, H=32, KVH=8, HD=128) on 8 TRN2
NeuronCores, tensor-parallel over heads.

Each core computes 4 query heads + their shared kv head end-to-end:
QKV projection -> RoPE -> causal attention (no-max-sub softmax, scores are
tiny) -> its slice of the wo projection. The 8 partial [S, D] outputs are
summed on the host (the "all-reduce after wo" of the sharding hint).

Device layouts (bf16/fp8 into the PE, fp32 PSUM accumulation):
  QT/KT  [HD=128(part), S]    from  lhsT=w[d,:], rhs=xT[d, s-tile]
  V      [S(part), HD]        via PE-transpose of VT
  scoresT[k(part), q]         lhsT=KT chunk, rhs=QT tile
  E = exp(scoresT/128) bf16; causal diagonal via 0/1 mask multiply
  attnT  [HD(part), q]        lhsT=V chunk, rhs=E  (accumulated over k)
  denom  [1, q]               lhsT=ones[128,1], rhs=E (accumulated over k)
  attnT_norm = attnT * bcast(1/denom)   (PE outer-product broadcast)
  out    [s(part), n]         wo in fp8 DoubleRow (head pairs as the
                              doubled contraction rows); the first 128
                              output rows redone in bf16 since early
                              causal rows have the largest magnitudes.

All x/w DRAM tensors are host-pre-swizzled so every DMA reads contiguous
>=4KB lines per partition. wo weights ride the vector DMA ring with their
triggers placed after phase A so they don't steal HBM bandwidth from the
x loads.
"""

import json
import math

import ml_dtypes
import numpy as np

import concourse.bass as bass
import concourse.tile as tile
from concourse import mybir
from concourse.bass_utils import run_bass_kernel_spmd

BF16 = mybir.dt.bfloat16
F32 = mybir.dt.float32
FP8 = mybir.dt.float8e4
NPBF16 = ml_dtypes.bfloat16
NPFP8 = ml_dtypes.float8_e4m3

# Full problem constants
B, S, D = 1, 2048, 4096
H, KVH = 32, 8
HD = 128
NCORES = 8
HQ = H // NCORES  # query heads per core
MULT = 1.0
ROPE_BASE = 10000.0
ST = 512  # s-tile (PSUM bank width in fp32)


def attn_scale(seq_len=S, d_head=HD, mult=MULT):
    alpha = 1.0 / (1.0 + 4.0 * d_head / mult**2)
    lower = (math.log(seq_len) / seq_len) ** 0.5
    interp = math.exp((1.0 - alpha) * math.log(lower))
    return 1.0 / interp


def _legalize_single_wait(nc):
    """The walrus build in this container accepts only ONE sync wait per
    instruction ("Too many sync wait commands" in setupSyncWait). Split
    extra waits into preceding single-wait Drains (lowered to CTRL NOPs)
    on the same engine — same in-order stall semantics."""
    bir = json.loads(nc.to_json_bytes())
    ctr = 0
    for fn in bir["functions"]:
        for blk in fn["blocks"]:
            out = []
            for inst in blk["instructions"]:
                si = inst.get("sync_info")
                waits = (si or {}).get("on_wait") or []
                if len(waits) > 1:
                    for w in waits[:-1]:
                        ctr += 1
                        out.append(
                            {
                                "debug": inst.get("debug", 0),
                                "engine": inst["engine"],
                                "ins": [],
                                "name": f"{inst['name']}-mw{ctr}",
                                "opcode": "Drain",
                                "outs": [],
                                "sync_info": {"on_update": [], "on_wait": [w]},
                            }
                        )
                    si["on_wait"] = [waits[-1]]
                out.append(inst)
            blk["instructions"] = out
    fixed = json.dumps(bir).encode()
    nc.to_json_bytes = lambda: fixed
    return nc


def build_core_kernel(s=S, d=D, hq=HQ):
    """Bass module for one core: hq query heads + 1 kv head."""
    nst = s // ST  # s-tiles of 512
    ndk = d // 128  # contraction chunks
    nh = hq + 2  # q heads + k + v
    nnt = d // ST  # output n-tiles

    nqk = hq + 1  # q heads + k (rope'd)
    nqkv = hq + 2  # q + k + v, all through the fp8 DoubleRow path
    npair = ndk // 2  # 256-row contraction pair-chunks (DoubleRow)
    nhp = hq // 2  # wo head pairs

    nc = bass.Bass()
    # host-preswizzled layouts (see host_prep): every slice a DMA pulls is
    # contiguous per partition
    x8_d = nc.dram_tensor("x8", [128, nst, 4, 4, 2, ST], FP8, kind="ExternalInput")
    xT_d = nc.dram_tensor("xT", [128, 4, 8, ST], BF16, kind="ExternalInput")
    w8_d = nc.dram_tensor("w8", [8, 128, 2, 2, nqkv * 128], FP8, kind="ExternalInput")
    wv_d = nc.dram_tensor("wv", [4, 128, 8, 128], BF16, kind="ExternalInput")
    wo8_d = nc.dram_tensor("wo8", [nhp, 128, 2, d], FP8, kind="ExternalInput")
    woT_d = nc.dram_tensor("woT", [hq * 128, d], BF16, kind="ExternalInput")
    cosF_d = nc.dram_tensor("cosF", [128, s], BF16, kind="ExternalInput")
    sinSg_d = nc.dram_tensor("sinSg", [128, s], BF16, kind="ExternalInput")
    maskT_d = nc.dram_tensor("maskT", [128, 128], BF16, kind="ExternalInput")
    mask4_d = nc.dram_tensor("mask4", [128, 4, ST], BF16, kind="ExternalInput")
    onesc8_d = nc.dram_tensor("onesc8", [128, 1], FP8, kind="ExternalInput")
    ident_d = nc.dram_tensor("ident", [128, 128], BF16, kind="ExternalInput")
    onesc_d = nc.dram_tensor("onesc", [128, 1], BF16, kind="ExternalInput")
    onesr_d = nc.dram_tensor("onesr", [128, 128], BF16, kind="ExternalInput")
    outp_d = nc.dram_tensor("outp", [s, d], BF16, kind="ExternalOutput")

    with tile.TileContext(nc) as tc:
        with (
            tc.tile_pool(name="const", bufs=1) as cp,
            tc.tile_pool(name="qkvsb", bufs=1) as qp,
            tc.tile_pool(name="xp", bufs=3) as xp,
            tc.tile_pool(name="xb", bufs=2) as xb,
            tc.tile_pool(name="rp", bufs=2) as rp,
            tc.tile_pool(name="vp", bufs=2) as vp,
            tc.tile_pool(name="ep", bufs=6) as ep,
            tc.tile_pool(name="ep8", bufs=3) as ep8,
            tc.tile_pool(name="sp", bufs=2) as sp,
            tc.tile_pool(name="op", bufs=5) as op,
            tc.tile_pool(name="at", bufs=2) as atp,
        ):
            # ---- resident constants (gpsimd DMA ring, needed-first order) ----
            w8g = [
                cp.tile([128, 2, 2, nqkv * 128], FP8, tag=f"w8{g}", name=f"w8{g}")
                for g in range(8)
            ]
            wvsb4 = [
                cp.tile([128, 8, 128], BF16, tag=f"wv{g}", name=f"wv{g}")
                for g in range(4)
            ]
            for g4 in range(4):
                nc.gpsimd.dma_start(w8g[2 * g4], w8_d[2 * g4])
                nc.gpsimd.dma_start(w8g[2 * g4 + 1], w8_d[2 * g4 + 1])
                nc.gpsimd.dma_start(wvsb4[g4], wv_d[g4])
            w8 = [w8g[j // 2][:, j % 2] for j in range(npair)]
            wvsb = [wvsb4[dk // 8][:, dk % 8, :] for dk in range(ndk)]
            cossb = cp.tile([128, s], BF16)
            nc.gpsimd.dma_start(cossb, cosF_d[:])
            sinsb = cp.tile([128, s], BF16)
            nc.gpsimd.dma_start(sinsb, sinSg_d[:])
            masksb = cp.tile([128, 128], BF16)
            nc.gpsimd.dma_start(masksb, maskT_d[:])
            mask4sb = cp.tile([128, 4, ST], BF16)
            nc.gpsimd.dma_start(mask4sb, mask4_d[:])
            onesc8sb = cp.tile([128, 1], FP8)
            nc.gpsimd.dma_start(onesc8sb, onesc8_d[:])
            identsb = cp.tile([128, 128], BF16)
            nc.gpsimd.dma_start(identsb, ident_d[:])
            onescsb = cp.tile([128, 1], BF16)
            nc.gpsimd.dma_start(onescsb, onesc_d[:])
            onescbsb = cp.tile([128, 128], BF16)
            nc.gpsimd.dma_start(onescbsb, onesr_d[:])
            # wo weights: tiles declared here, DMAs deferred to phase B entry
            wo8sb = [
                cp.tile([128, 2, d], FP8, tag=f"wo8{hp}", name=f"wo8{hp}")
                for hp in range(nhp)
            ]
            wosb = [
                cp.tile([128, d], BF16, tag=f"wo{mh}", name=f"wo{mh}")
                for mh in range(hq)
            ]

            warm = cp.tile([128, ST], BF16, tag="warm")
            nc.vector.memset(warm, 0.0)

            # ---- persistent activations (bf16), one tile per s-tile so a
            # q-tile's attention only depends on the s-tiles it reads ----
            qts_sb = [
                [
                    qp.tile([128, ST], BF16, tag=f"QT{h}_{st}", name=f"QT{h}_{st}")
                    for st in range(nst)
                ]
                for h in range(hq)
            ]
            kts_sb = [
                qp.tile([128, ST], BF16, tag=f"KT{st}", name=f"KT{st}")
                for st in range(nst)
            ]
            # bf16 V for s-tile 0 only (feeds qt0's accurate AV path)
            v0_sb = qp.tile([128, ST], BF16, tag="V0", name="V0")
            # fp8 V chunk-pairs for the DoubleRow AV path of qt1-3:
            # [s%128, pair-in-stile, chunk-parity, hd]
            vs8_sb = [
                qp.tile([128, 2, 2, 128], FP8, tag=f"V8{st}", name=f"V8{st}")
                for st in range(nst)
            ]

            # ================= phase A: QKV projection + RoPE =================
            with (
                tc.tile_pool(name="psA", bufs=7, space="PSUM") as psA,
                tc.tile_pool(name="psT", bufs=1, space="PSUM") as psT,
            ):
                nq = 4  # quarters per s-tile
                ndkq = ndk // nq  # bf16 contraction chunks per quarter (V)
                npq = npair // nq  # fp8 pair-chunks per quarter (QK)

                def emit_rope(st, stg):
                    ssl = slice(st * ST, (st + 1) * ST)
                    for h in [hq] + list(range(hq)):
                        dst = qts_sb[h][st] if h < hq else kts_sb[st]
                        t1 = rp.tile([128, ST], BF16, tag="t1", name=f"t1_{st}_{h}")
                        nc.vector.tensor_mul(t1, stg[h], cossb[:, ssl])
                        tsw = rp.tile([128, ST], BF16, tag="tsw", name=f"tsw_{st}_{h}")
                        nc.vector.tensor_copy(tsw[0:64, :], stg[h][64:128, :])
                        nc.vector.tensor_copy(tsw[64:128, :], stg[h][0:64, :])
                        nc.vector.tensor_mul(tsw, tsw, sinsb[:, ssl])
                        nc.vector.tensor_add(dst, t1, tsw)

                rope_tail = None
                # warmup matmuls on a zeroed tile: no input deps, so they run
                # during the initial DMA wait, keeping the PE busy from ~6us
                # and the HAM clock-gate at full rate when real work lands
                wm_ps = psA.tile([128, ST], F32, tag="acc", name="warmps")
                for _ in range(38):
                    nc.tensor.matmul(wm_ps, warm[:, 0:128], warm, start=True, stop=True)
                for st in range(nst):
                    ssl = slice(st * ST, (st + 1) * ST)
                    acc = [
                        psA.tile([128, ST], F32, tag="acc", name=f"acc{h}")
                        for h in range(nqkv)
                    ]
                    if st == 0:
                        acc_vb = psA.tile([128, ST], F32, tag="acc", name="accvb")
                    # heads-major over resident xT quarters: at the next s-tile
                    # boundary only acc[0] must be free for PE to proceed
                    for quar in range(nq):
                        x8a = xp.tile([128, npq, 2, ST], FP8, tag="x8")
                        if st == 0 and quar == 0:
                            # split the very first load so the first matmul's
                            # slice lands sooner
                            nc.sync.dma_start(x8a[:, 0:2], x8_d[:, st, quar, 0:2])
                            nc.sync.dma_start(x8a[:, 2:4], x8_d[:, st, quar, 2:4])
                        else:
                            nc.sync.dma_start(x8a, x8_d[:, st, quar])
                        # Q + K + V: fp8 DoubleRow, 256-deep contraction per mm
                        for h in range(nqkv):
                            for i in range(npq):
                                nc.tensor.matmul(
                                    acc[h],
                                    w8[quar * npq + i][:, :, h * 128 : (h + 1) * 128],
                                    x8a[:, i, :, :],
                                    start=(quar == 0 and i == 0),
                                    stop=(quar == nq - 1 and i == npq - 1),
                                    perf_mode=mybir.MatmulPerfMode.DoubleRow,
                                )
                        if st == 0:
                            # accurate bf16 V for the first s-tile: its rows
                            # feed the first 512 output rows, where magnitudes
                            # are largest and the fp8 V error would show
                            xta = xb.tile([128, ndkq, ST], BF16, tag="xT")
                            nc.scalar.dma_start(xta, xT_d[:, quar])
                            for dk in range(ndkq):
                                nc.tensor.matmul(
                                    acc_vb,
                                    wvsb[quar * ndkq + dk],
                                    xta[:, dk, :],
                                    start=(quar == 0 and dk == 0),
                                    stop=(quar == nq - 1 and dk == ndkq - 1),
                                )
                    # V: transpose [HD, s-tile] -> [s-chunk, HD] blocks.
                    # Emitted before RoPE so the PE's transposes don't wait
                    # behind the RoPE chain on the DVE.
                    for j in range(ST // 128):
                        vtmp = vp.tile([128, 128], BF16, tag="vtmp")
                        nc.scalar.copy(vtmp, (acc_vb if st == 0 else acc[hq + 1])[:, j * 128 : (j + 1) * 128])
                        tp_ps = psT.tile([128, 128], BF16, tag="tp")
                        nc.tensor.transpose(tp_ps, vtmp, identsb)
                        nc.vector.tensor_copy(
                            vs8_sb[st][:, j // 2, j % 2, :], tp_ps
                        )
                        if st == 0:
                            nc.vector.tensor_copy(
                                v0_sb[:, j * 128 : (j + 1) * 128], tp_ps
                            )
                    # stage QK accumulators to bf16 (one ACT copy each, frees
                    # the PSUM bank early); RoPE then runs in bf16 on the DVE
                    stg = {}
                    for h in range(hq + 1):
                        stg[h] = rp.tile(
                            [128, ST], BF16, tag=f"stg{h}", name=f"stg{st}_{h}"
                        )
                        nc.scalar.copy(stg[h], acc[h])
                    if st < nst - 1:
                        emit_rope(st, stg)
                    else:
                        rope_tail = (st, stg)

            # wo weight DMAs ride the sync ring behind all 16 x8 loads
            # (rings are FIFO, and the x8 triggers themselves gate on tile
            # reuse), so wo traffic starts only near the end of phase A and
            # can't steal HBM bandwidth from the x loads
            for hp in range(nhp):
                nc.sync.dma_start(wo8sb[hp], wo8_d[hp])
            for mh in range(hq):
                nc.sync.dma_start(wosb[mh], woT_d[mh * 128 : (mh + 1) * 128, :])

            # ============ phase B: attention + output projection ============
            with (
                tc.tile_pool(name="psS", bufs=3, space="PSUM") as psS,
                tc.tile_pool(name="psD", bufs=1, space="PSUM") as psD,
                tc.tile_pool(name="psAt", bufs=4, space="PSUM") as psAt,
            ):

                def emit_wo(qt, attn8, attnb, lo=0, hi=16):
                    # wo units [lo:hi) for q-tile qt; a unit is one s-chunk x
                    # two n-tiles -> one 2KB-per-partition out DMA. Units are
                    # interleaved into the NEXT q-tile's attention chunks so
                    # the out-DMA bandwidth is spread across the whole window
                    # instead of bursting (8 cores bursting saturates HBM
                    # writes). fp8 DoubleRow over head pairs; the first 128
                    # rows (largest magnitudes, short span) redone in bf16.
                    # Both halves of a pair are copied by the SAME engine so
                    # the DMA has a single producer; engines and DMA rings
                    # alternate per unit.
                    with nc.named_scope(f"wo{qt}"):
                        for u in range(lo, hi):
                            j, ntg = u // (nnt // 2), u % (nnt // 2)
                            sc = qt * (ST // 128) + j
                            osb = op.tile(
                                [128, 2 * ST], BF16, tag="osb",
                                name=f"osb{qt}_{j}_{ntg}",
                            )

                            for half in range(2):
                                nt = 2 * ntg + half
                                o_ps = psS.tile(
                                    [128, ST], F32, tag="sc",
                                    name=f"wo{qt}_{j}_{nt}",
                                )
                                if qt == 0 and j == 0:
                                    for mh in range(hq):
                                        nc.tensor.matmul(
                                            o_ps,
                                            attnb[mh],
                                            wosb[mh][:, nt * ST : (nt + 1) * ST],
                                            start=(mh == 0),
                                            stop=(mh == hq - 1),
                                        )
                                else:
                                    for hp in range(nhp):
                                        nc.tensor.matmul(
                                            o_ps,
                                            attn8[hp][:, :, j * 128 : (j + 1) * 128],
                                            wo8sb[hp][:, :, nt * ST : (nt + 1) * ST],
                                            start=(hp == 0),
                                            stop=(hp == nhp - 1),
                                            perf_mode=mybir.MatmulPerfMode.DoubleRow,
                                        )
                                dst = osb[:, half * ST : (half + 1) * ST]
                                if u % 4 != 3:
                                    nc.vector.tensor_copy(dst, o_ps)
                                else:
                                    nc.scalar.copy(dst, o_ps)
                            ring = nc.sync if u % 4 != 3 else nc.gpsimd
                            ring.dma_start(
                                outp_d[
                                    sc * 128 : (sc + 1) * 128,
                                    2 * ntg * ST : (2 * ntg + 2) * ST,
                                ],
                                osb,
                            )

                prev_wo = None
                for qt in range(nst):
                    nk = (qt + 1) * (ST // 128)  # causal: k chunks this q-tile
                    emitted = 0
                    with nc.named_scope(f"attn{qt}"):
                        # one denominator bank per q-tile: head h accumulates
                        # into partition row 32*h (distinct col-groups)
                        den4 = psD.tile([128, ST], F32, tag="den")
                        nc.vector.memset(den4, 1.0)
                        at_tiles = {
                            h: psAt.tile([128, ST], F32, tag="at", name=f"at{qt}_{h}")
                            for h in range(hq)
                        }
                        npr = nk // 2  # chunk pairs (DoubleRow AV/den)
                        e8_pair = {}
                        for c in range(nk):
                            # diagonal chunks: only columns >= 128*r valid.
                            # qt0 computes narrow slices (bf16, accurate);
                            # qt1-3 compute full width and zero the invalid
                            # region via mask so chunk-pairs stay aligned for
                            # the fp8 DoubleRow AV path.
                            r = c - (nk - 4)
                            if qt == 0:
                                off = 128 * r if r > 0 else 0
                            else:
                                off = 0
                            w = ST - off
                            e_ts = {}
                            for h in range(hq):
                                sc_ps = psS.tile(
                                    [128, ST], F32, tag="sc", name=f"sc{qt}_{c}_{h}"
                                )
                                nc.tensor.matmul(
                                    sc_ps[:, 0:w],
                                    kts_sb[c // 4][:, (c % 4) * 128 : (c % 4 + 1) * 128],
                                    qts_sb[h][qt][:, off:ST],
                                    start=True,
                                    stop=True,
                                )
                                if qt == 0:
                                    e_t = ep.tile(
                                        [128, ST], BF16, tag="E", name=f"e{qt}_{c}_{h}"
                                    )
                                    nc.scalar.activation(
                                        e_t[:, 0:w],
                                        sc_ps[:, 0:w],
                                        mybir.ActivationFunctionType.Exp,
                                        scale=1.0 / HD,
                                    )
                                    if r >= 0:
                                        nc.vector.tensor_mul(
                                            e_t[:, 0:128], e_t[:, 0:128], masksb
                                        )
                                    e_ts[h] = e_t
                                else:
                                    if c % 2 == 0:
                                        e8_pair[h] = ep8.tile(
                                            [128, 2, ST], FP8, tag=f"E8{h}",
                                            name=f"e8_{qt}_{c}_{h}",
                                        )
                                    dst8 = e8_pair[h][:, c % 2, :]
                                    if r >= 0:
                                        e_t = ep.tile(
                                            [128, ST], BF16, tag="E",
                                            name=f"e{qt}_{c}_{h}",
                                        )
                                        nc.scalar.activation(
                                            e_t,
                                            sc_ps,
                                            mybir.ActivationFunctionType.Exp,
                                            scale=1.0 / HD,
                                        )
                                        nc.vector.tensor_mul(
                                            dst8, e_t, mask4sb[:, r, :]
                                        )
                                    else:
                                        nc.scalar.activation(
                                            dst8,
                                            sc_ps,
                                            mybir.ActivationFunctionType.Exp,
                                            scale=1.0 / HD,
                                        )
                            if qt == 0:
                                for h in range(hq):
                                    nc.tensor.matmul(
                                        at_tiles[h][:, off:ST],
                                        v0_sb[:, c * 128 : (c + 1) * 128],
                                        e_ts[h][:, 0:w],
                                        start=(c == 0),
                                        stop=(c == nk - 1),
                                    )
                                for h in range(hq):
                                    nc.tensor.matmul(
                                        den4[32 * h : 32 * h + 1, off:ST],
                                        onescsb,
                                        e_ts[h][:, 0:w],
                                        start=(c == 0),
                                        stop=(c == nk - 1),
                                        tile_position=(0, 32 * h),
                                    )
                            elif c % 2 == 1:
                                g = c // 2
                                for h in range(hq):
                                    nc.tensor.matmul(
                                        at_tiles[h],
                                        vs8_sb[g // 2][:, g % 2],
                                        e8_pair[h],
                                        start=(g == 0),
                                        stop=(g == npr - 1),
                                        perf_mode=mybir.MatmulPerfMode.DoubleRow,
                                    )
                                for h in range(hq):
                                    for ko in range(2):
                                        nc.tensor.matmul(
                                            den4[32 * h : 32 * h + 1, :],
                                            onesc8sb,
                                            e8_pair[h][:, ko, :],
                                            start=(g == 0 and ko == 0),
                                            stop=(g == npr - 1 and ko == 1),
                                            tile_position=(0, 32 * h),
                                        )
                            # spread 12 of prev's 16 wo units across this
                            # q-tile's chunks; hold 4 back to cover the
                            # reciprocal latency after the last denominator
                            if prev_wo is not None and c < nk - 1:
                                target = (12 * (c + 1)) // nk
                                if target > emitted:
                                    emit_wo(*prev_wo, emitted, target)
                                    emitted = target
                    # previous q-tile's remaining wo matmuls fill the PE
                    # while this q-tile's reciprocal tail runs on the DVE
                    if prev_wo is not None:
                        emit_wo(*prev_wo, emitted, 16)
                    with nc.named_scope(f"norm{qt}"):
                        # reciprocal as exp(-ln(x)) on the Scalar engine:
                        # ~1.2us latency instead of 3.4us on the DVE, and no
                        # DVE-queue coupling; den4 is in [1, ~3e3] so ln is
                        # well-conditioned and bf16 output absorbs the error
                        lnden = sp.tile([128, ST], F32, tag="recip", name=f"lnden{qt}")
                        nc.scalar.activation(
                            lnden, den4, mybir.ActivationFunctionType.Ln
                        )
                        recipb = sp.tile([128, ST], BF16, tag="recipb", name=f"recipb{qt}")
                        nc.scalar.activation(
                            recipb, lnden, mybir.ActivationFunctionType.Exp,
                            scale=-1.0,
                        )
                        attn8 = [
                            atp.tile([128, 2, ST], FP8, tag=f"attn8{hp}",
                                     name=f"attn8_{qt}_{hp}")
                            for hp in range(nhp)
                        ]
                        attnb = None
                        if qt == 0:
                            attnb = [
                                atp.tile([128, 128], BF16, tag=f"attnb{mh}",
                                         name=f"attnb{mh}")
                                for mh in range(hq)
                            ]
                        for hh in range(hq):
                            bc_ps = psS.tile(
                                [128, ST], F32, tag="sc", name=f"bc{qt}_{hh}"
                            )
                            nc.tensor.matmul(
                                bc_ps,
                                onescbsb[32 * hh : 32 * hh + 1, :],
                                recipb[32 * hh : 32 * hh + 1, :],
                                start=True,
                                stop=True,
                                tile_position=(32 * hh, 0),
                            )
                            bc_sb = sp.tile(
                                [128, ST], F32, tag="bcsb", name=f"bcsb{qt}_{hh}"
                            )
                            nc.scalar.copy(bc_sb, bc_ps)
                            nc.vector.tensor_mul(
                                attn8[hh // 2][:, hh % 2, :], at_tiles[hh], bc_sb
                            )
                            if qt == 0:
                                nc.vector.tensor_mul(
                                    attnb[hh], at_tiles[hh][:, 0:128],
                                    bc_sb[:, 0:128],
                                )
                    prev_wo = (qt, attn8, attnb)
                    if qt == 0 and rope_tail is not None:
                        emit_rope(*rope_tail)
                emit_wo(*prev_wo)
    return _legalize_single_wait(nc)


def host_prep(x, wq, wk, wv, wo, s=S, d=D, hq=HQ, ncores=NCORES):
    """Shared tensors + per-core weight shards, all host-side numpy.
    Everything is pre-swizzled into the exact on-chip tile layouts so each
    DMA slice reads contiguous per-partition lines."""
    scale = attn_scale(s, HD, MULT)
    nst = s // ST
    xTf = np.ascontiguousarray(x.reshape(s, d).T)  # [d, s]
    # xT (s-tile 0 only, for the accurate bf16 V of the first 512 rows)
    # [p, quar, dk8, n] <- xT[(quar*8+dk8)*128+p, n]
    xT = np.ascontiguousarray(
        xTf[:, 0:ST].astype(NPBF16).reshape(4, 8, 128, ST).transpose(2, 0, 1, 3)
    )
    # x8 [p, st, quar, i4, ko, n] <- x8[(quar*4+i4)*256+ko*128+p, st*512+n]
    x8 = np.ascontiguousarray(
        xTf.astype(NPFP8).reshape(4, 4, 2, 128, nst, ST).transpose(3, 4, 0, 1, 2, 5)
    )

    freq = ROPE_BASE ** (-(np.arange(0, HD, 2, dtype=np.float64) / HD))
    pos = np.arange(s, dtype=np.float64)
    angle = pos[:, None] * freq[None, :]  # [s, 64]
    cos = np.cos(angle).astype(NPBF16).T  # [64, s]
    sin = np.sin(angle).astype(NPBF16).T
    cosF = np.ascontiguousarray(np.concatenate([cos, cos], axis=0))
    sinSg = np.ascontiguousarray(np.concatenate([-sin, sin], axis=0))

    # triangular causal mask for diagonal chunks: keep iff p <= f
    p = np.arange(128)[:, None]
    f = np.arange(128)[None, :]
    maskT = (p <= f).astype(NPBF16)  # [128, 128]

    # full-width diagonal masks for qt>=1: valid iff r*128 + p <= f
    pp = np.arange(128)[:, None, None]
    rr = np.arange(4)[None, :, None]
    ff = np.arange(ST)[None, None, :]
    mask4 = (rr * 128 + pp <= ff).astype(NPBF16)  # [128, 4, ST]
    onesc8 = np.ones((128, 1), dtype=NPFP8)

    ident = np.eye(128, dtype=NPBF16)
    onesc = np.ones((128, 1), dtype=NPBF16)
    onesr = np.ones((128, 128), dtype=NPBF16)

    shared = dict(
        xT=xT, x8=x8, cosF=cosF, sinSg=sinSg, maskT=maskT, mask4=mask4,
        onesc8=onesc8, ident=ident, onesc=onesc, onesr=onesr,
    )

    in_maps = []
    for c in range(ncores):
        wq_c = wq[c * hq * 128 : (c + 1) * hq * 128, :]  # [hq*128, d]
        wk_c = wk[c * 128 : (c + 1) * 128, :]
        wv_c = wv[c * 128 : (c + 1) * 128, :] * scale
        wqk8f = np.ascontiguousarray(
            np.concatenate([wq_c.T, wk_c.T, wv_c.T], axis=1)
        ).astype(NPFP8)  # [d, (hq+2)*128]
        # w8 [g, p, j2, ko, m] <- wqk8[((g*2+j2)*2+ko)*128+p, m]
        w8 = np.ascontiguousarray(
            wqk8f.reshape(8, 2, 2, 128, (hq + 2) * 128).transpose(0, 3, 1, 2, 4)
        )
        # wv [g, p, c8, n] <- wvT[(g*8+c8)*128+p, n]
        wvT = np.ascontiguousarray(
            wv_c.T.astype(NPBF16).reshape(4, 8, 128, 128).transpose(0, 2, 1, 3)
        )
        wo_c = wo[:, c * hq * 128 : (c + 1) * hq * 128]  # [d, hq*128]
        woT = np.ascontiguousarray(wo_c.T)  # [hq*128, d] f32
        # wo8 [hp, ki, ko, n] <- woT[(2hp+ko)*128+ki, n]
        wo8 = np.ascontiguousarray(
            woT.astype(NPFP8).reshape(hq // 2, 2, 128, d).transpose(0, 2, 1, 3)
        )
        in_maps.append(
            dict(shared, w8=w8, wv=wvT, wo8=wo8, woT=woT.astype(NPBF16))
        )
    return in_maps


_NC_CACHE = {}


def kernel(x, freqs_cis, wq, wk, wv, wo):
    del freqs_cis  # forward pass recomputes rope tables (matches reference)
    x = np.asarray(x, dtype=np.float32)
    key = (S, D, HQ)
    if key not in _NC_CACHE:
        _NC_CACHE[key] = build_core_kernel(S, D, HQ)
    nc = _NC_CACHE[key]
    in_maps = host_prep(
        x, np.asarray(wq, np.float32), np.asarray(wk, np.float32),
        np.asarray(wv, np.float32), np.asarray(wo, np.float32),
    )
    res = run_bass_kernel_spmd(nc, in_maps, core_ids=list(range(NCORES)))
    out = np.zeros((S, D), dtype=np.float32)
    for r in res.results:
        out += np.asarray(r["outp"], dtype=np.float32)
    return out.reshape(B, S, D)


if __name__ == "__main__":
    rng = np.random.default_rng(0)
    x = rng.standard_normal((B, S, D)).astype(np.float32)
    wq = (rng.standard_normal((H * HD, D)) * D**-0.5).astype(np.float32)
    wk = (rng.standard_normal((KVH * HD, D)) * D**-0.5).astype(np.float32)
    wv = (rng.standard_normal((KVH * HD, D)) * D**-0.5).astype(np.float32)
    wo = (rng.standard_normal((D, H * HD)) * (H * HD) ** -0.5).astype(np.float32)
    fc = rng.standard_normal((S, HD // 2)).astype(np.float32)
    out = kernel(x, fc, wq, wk, wv, wo)
    print(out.shape, out.dtype, np.abs(out).max())
